# revision 1
# baseline (speedup 1.0000x reference)
"""Trainium2 Bass kernel for nn_DGEBlock (dense transformer block with
MoE-gated linears), distributed over 8 NeuronCores.

Sharding: data-parallel over batch (2 groups of 4 cores) x sequence-parallel
over tokens within each batch (512 tokens per core). Weights are replicated
(bf16, host pre-tiled); activations live feature-major ("T-layout": [d, tok])
in SBUF so projections are lhsT=W^T-tile @ rhs=activation with no activation
transposes. V is projected in token-major (N-)layout directly so attention's
PV matmuls need no transposes either. The only collectives are two 4-rank
AllGathers (V then K, bf16); their dependent loads are issued from the
GpSimd/Vector queues so they never head-of-line-block the Sync queue's
weight streaming. Output is returned token-sharded and reassembled on host.
"""

import sys

for _p in ("/opt/trn_rl_repo",):
    if _p not in sys.path:
        sys.path.append(_p)

import numpy as np
import ml_dtypes

# ---------------------------------------------------------------- constants
B = 2
T = 2048
D = 2048
H = 16
HD = 128
FF = 4 * D  # 8192
EPS = 1e-5

N_CORES = 8
GROUP = 4  # cores per batch group (sequence-parallel degree)
S = T // GROUP  # tokens per core = 512
P = 128
NT = D // P  # 16 feature tiles
NF = FF // P  # 64 hidden tiles
NKB = T // P  # 16 key blocks per batch
ISCALE = 1.0 / float(np.sqrt(HD))

RG = [[0, 1, 2, 3], [4, 5, 6, 7]]

_BF = ml_dtypes.bfloat16

_COMPILED = None


# ------------------------------------------------------------- host prep
def _w_tiled(W):
    """W [dout, din] -> [nj, 128, nt, 128] bf16 such that
    out[j, p, t, jc] == W[j*128+jc, t*128+p]  (= W^T tile (t, j)).
    Per (j, partition p) the free dims (t, jc) are contiguous in memory."""
    dout, din = W.shape
    nj, nt = dout // P, din // P
    return np.ascontiguousarray(
        W.reshape(nj, P, nt, P).transpose(0, 3, 2, 1).astype(_BF)
    )


def _b_cols(b):
    """b [dout] -> [128, nj] fp32: column j holds b[j*128:(j+1)*128]."""
    nj = b.shape[0] // P
    return np.ascontiguousarray(b.reshape(nj, P).T.astype(np.float32))


# ------------------------------------------------------------- device build
def _build():
    from concourse import bacc, tile, mybir

    fp32 = mybir.dt.float32
    bf16 = mybir.dt.bfloat16
    AF = mybir.ActivationFunctionType
    ALU = mybir.AluOpType

    nc = bacc.Bacc("TRN2", target_bir_lowering=False, debug=False,
                   num_devices=N_CORES)

    # ---- I/O tensors
    xT_d = nc.dram_tensor("xT", [D, S], fp32, kind="ExternalInput")
    wd = {}
    for nm in ("Wq", "Wgq", "Wk", "Wgk", "Wo", "Wgo"):
        wd[nm] = nc.dram_tensor(nm, [NT, P, NT, P], bf16, kind="ExternalInput")
    for nm in ("Win", "Wgin"):
        wd[nm] = nc.dram_tensor(nm, [NF, P, NT, P], bf16, kind="ExternalInput")
    for nm in ("Wout", "Wgout"):
        wd[nm] = nc.dram_tensor(nm, [NT, P, NF, P], bf16, kind="ExternalInput")
    # V projection runs in N-layout: plain W^T [din, dout] bf16 + bias rows
    wd["WvT"] = nc.dram_tensor("WvT", [D, D], bf16, kind="ExternalInput")
    wd["WgvT"] = nc.dram_tensor("WgvT", [D, D], bf16, kind="ExternalInput")
    bvrow_d = nc.dram_tensor("bvrow", [1, D], bf16, kind="ExternalInput")
    bgvrow_d = nc.dram_tensor("bgvrow", [1, D], bf16, kind="ExternalInput")
    bd = {}
    for nm in ("bq", "bgq", "bk", "bgk", "bo", "bgo",
               "bout", "bgout", "g1", "bt1", "g2", "bt2"):
        bd[nm] = nc.dram_tensor(nm, [P, NT], fp32, kind="ExternalInput")
    for nm in ("bin", "bgin"):
        bd[nm] = nc.dram_tensor(nm, [P, NF], fp32, kind="ExternalInput")
    out_d = nc.dram_tensor("outT", [D, S], fp32, kind="ExternalOutput")

    with tile.TileContext(nc) as tc:
        with (
            tc.tile_pool(name="const", bufs=1) as constp,
            tc.tile_pool(name="bias", bufs=1) as biasp,
            tc.tile_pool(name="rows", bufs=1) as rows,
            tc.tile_pool(name="dram", bufs=1, space="DRAM") as dramp,
        ):
            ones_col = constp.tile([P, 1], bf16)
            nc.vector.memset(ones_col[:], 1.0)
            ones_row = constp.tile([1, P], bf16)
            nc.vector.memset(ones_row[:], 1.0)
            eps_t = constp.tile([1, 1], fp32)
            nc.vector.memset(eps_t[:], EPS)
            bvrow = constp.tile([1, D], bf16)
            nc.sync.dma_start(bvrow[:], bvrow_d.ap())
            bgvrow = constp.tile([1, D], bf16)
            nc.sync.dma_start(bgvrow[:], bgvrow_d.ap())

            bias = {}
            for nm in bd:
                ncols = NF if nm in ("bin", "bgin") else NT
                btile = biasp.tile([P, ncols], fp32, name=f"bias_{nm}")
                nc.sync.dma_start(btile[:], bd[nm].ap())
                bias[nm] = btile

            # ---------- helpers ----------
            def ln_T(src, gname, bname, hpool, tmpool, psln, name):
                """LayerNorm over the feature dim of a T-layout activation.

                src: SBUF tile [128, NT, S] fp32 -> returns bf16 [128, NT, S].
                Stats via ones-matmuls (contract over partitions); per-token
                scale/shift rows are broadcast to [128, S] via rank-1 matmuls.
                Row chain kept on DVE (ACT only for Sqrt / Square / the final
                per-tile affine) to minimize engine hops and table reloads.
                """
                xbf = tmpool.tile([P, NT, S], bf16, name=f"{name}_xbf")
                sq = tmpool.tile([P, NT, S], bf16, name=f"{name}_sq")
                for t in range(NT):
                    nc.vector.tensor_copy(xbf[:, t, :], src[:, t, :])
                    nc.scalar.activation(sq[:, t, :], src[:, t, :], AF.Square)
                S1 = psln.tile([1, S], fp32, name=f"{name}_S1", tag="ln_S1")
                S2 = psln.tile([1, S], fp32, name=f"{name}_S2", tag="ln_S2")
                for t in range(NT):
                    nc.tensor.matmul(S1[:], ones_col[:], xbf[:, t, :],
                                     start=(t == 0), stop=(t == NT - 1))
                for t in range(NT):
                    nc.tensor.matmul(S2[:], ones_col[:], sq[:, t, :],
                                     start=(t == 0), stop=(t == NT - 1))

                def row(nm, dt=fp32):
                    return rows.tile([1, S], dt, name=f"{name}_{nm}",
                                     tag=f"ln_{nm}")

                mean = row("mean")
                nc.vector.tensor_scalar_mul(mean[:], S1[:], 1.0 / D)
                m2 = row("m2")
                nc.vector.tensor_scalar_mul(m2[:], S2[:], 1.0 / D)
                msq = row("msq")
                nc.vector.tensor_tensor(msq[:], mean[:], mean[:],
                                        op=ALU.mult)
                var = row("var")
                nc.vector.tensor_tensor(var[:], m2[:], msq[:],
                                        op=ALU.subtract)
                std = row("std")
                nc.scalar.activation(std[:], var[:], AF.Sqrt,
                                     bias=eps_t[:])
                rstd = row("rstd")
                nc.vector.reciprocal(rstd[:], std[:])
                rstd_bf = row("rstdbf", bf16)
                nc.vector.tensor_copy(rstd_bf[:], rstd[:])
                mr_bf = row("mrbf", bf16)
                nc.vector.tensor_tensor(mr_bf[:], mean[:], rstd[:],
                                        op=ALU.mult)
                Ab_p = psln.tile([P, S], fp32, name=f"{name}_Abp",
                                 tag="ln_Abp")
                nc.tensor.matmul(Ab_p[:], ones_row[:], rstd_bf[:])
                Bb_p = psln.tile([P, S], fp32, name=f"{name}_Bbp",
                                 tag="ln_Bbp")
                nc.tensor.matmul(Bb_p[:], ones_row[:], mr_bf[:])
                Ab = tmpool.tile([P, S], fp32, name=f"{name}_Ab")
                nc.vector.tensor_copy(Ab[:], Ab_p[:])
                Bb = tmpool.tile([P, S], fp32, name=f"{name}_Bb")
                nc.vector.tensor_copy(Bb[:], Bb_p[:])
                h = hpool.tile([P, NT, S], bf16, name=f"{name}_h")
                for t in range(NT):
                    tmp = tmpool.tile([P, S], fp32, name=f"{name}_t0_{t}",
                                      tag="ln_t0", bufs=3)
                    nc.vector.tensor_tensor(tmp[:], src[:, t, :], Ab[:],
                                            op=ALU.mult)
                    tmp2 = tmpool.tile([P, S], fp32, name=f"{name}_t1_{t}",
                                       tag="ln_t1", bufs=3)
                    nc.vector.tensor_tensor(tmp2[:], tmp[:], Bb[:],
                                            op=ALU.subtract)
                    nc.scalar.activation(h[:, t, :], tmp2[:], AF.Identity,
                                         bias=bias[bname][:, t:t + 1],
                                         scale=bias[gname][:, t:t + 1])
                return h

            def proj_gated(src, nt, nj, wname, wgname, bgname, wpool,
                           pspool, epilogue, tchunk=None, wbufs=3):
                """Gated projection in T-layout: for each output tile j,
                main/gate = sum_t W^T(t,j).T @ src[:,t,:], then
                epilogue(j, main_psum, sig_sbuf)."""
                if tchunk is None:
                    tchunk = nt
                nchunk = nt // tchunk
                for j in range(nj):
                    main = pspool.tile([P, S], fp32, name=f"{wname}_m{j}",
                                       tag="pj_main", bufs=2)
                    gate = pspool.tile([P, S], fp32, name=f"{wname}_g{j}",
                                       tag="pj_gate", bufs=2)
                    for ci in range(nchunk):
                        wt = wpool.tile([P, tchunk, P], bf16, tag="wmain",
                                        name=f"w_{wname}_{j}_{ci}",
                                        bufs=wbufs)
                        nc.sync.dma_start(
                            wt[:],
                            wd[wname].ap()[j, :,
                                           ci * tchunk:(ci + 1) * tchunk, :])
                        for ti in range(tchunk):
                            t = ci * tchunk + ti
                            nc.tensor.matmul(main[:], wt[:, ti, :],
                                             src[:, t, :],
                                             start=(t == 0),
                                             stop=(t == nt - 1))
                    for ci in range(nchunk):
                        wg = wpool.tile([P, tchunk, P], bf16, tag="wgate",
                                        name=f"w_{wgname}_{j}_{ci}",
                                        bufs=wbufs)
                        nc.sync.dma_start(
                            wg[:],
                            wd[wgname].ap()[j, :,
                                            ci * tchunk:(ci + 1) * tchunk, :])
                        for ti in range(tchunk):
                            t = ci * tchunk + ti
                            nc.tensor.matmul(gate[:], wg[:, ti, :],
                                             src[:, t, :],
                                             start=(t == 0),
                                             stop=(t == nt - 1))
                    sig = wpool.tile([P, S], bf16, tag="sig",
                                     name=f"sig_{wname}_{j}", bufs=3)
                    nc.scalar.activation(sig[:], gate[:], AF.Sigmoid,
                                         bias=bias[bgname][:, j:j + 1])
                    epilogue(j, main, sig)

            # x2 outlives phases A-C (used by LN2 + MLP residual)
            with tc.tile_pool(name="x2p", bufs=1) as x2p:
              with tc.tile_pool(name="xt", bufs=1) as xtp:
                xt = xtp.tile([P, NT, S], fp32)
                xT_v = xT_d.ap().rearrange("(t p) s -> t p s", p=P)
                for t in range(NT):
                    nc.sync.dma_start(xt[:, t, :], xT_v[t])

                vN_bounce = dramp.tile([S, D], bf16)
                k_bounce = dramp.tile([D, S], bf16)
                vgN = dramp.tile([GROUP * S, D], bf16)
                kg = dramp.tile([GROUP * D, S], bf16)

                with tc.tile_pool(name="yp", bufs=1) as ypool:
                  with tc.tile_pool(name="qp", bufs=1) as qpool:
                    q = qpool.tile([P, NT, S], bf16)

                    with tc.tile_pool(name="hq", bufs=1) as hqp:
                        with (
                            tc.tile_pool(name="ln1tmp", bufs=1) as ln1tmp,
                            tc.tile_pool(name="ln1ps", bufs=1,
                                         space="PSUM") as ln1ps,
                        ):
                            h1 = ln_T(xt, "g1", "bt1", hqp, ln1tmp, ln1ps,
                                      "ln1")

                        # ---- V projection, N-layout (option i) ----
                        with (
                            tc.tile_pool(name="wv", bufs=1) as wvp,
                            tc.tile_pool(name="vps", bufs=1,
                                         space="PSUM") as vps,
                        ):
                            TC = NT // 2
                            for n in range(4):
                                vmain = [vps.tile([P, S], fp32,
                                                  tag="v_main", bufs=4,
                                                  name=f"vm_{n}_{m}")
                                         for m in range(4)]
                                vgate = [vps.tile([P, S], fp32,
                                                  tag="v_gate", bufs=4,
                                                  name=f"vg_{n}_{m}")
                                         for m in range(4)]
                                for ci in range(2):
                                    wvt = wvp.tile([P, TC, 4 * P], bf16,
                                                   tag="wv", bufs=2,
                                                   name=f"wv_{n}_{ci}")
                                    wgvt = wvp.tile([P, TC, 4 * P], bf16,
                                                    tag="wgv", bufs=2,
                                                    name=f"wgv_{n}_{ci}")
                                    for ti in range(TC):
                                        t = ci * TC + ti
                                        nc.sync.dma_start(
                                            wvt[:, ti, :],
                                            wd["WvT"].ap()[t * P:(t + 1) * P,
                                                           n * S:(n + 1) * S])
                                        nc.sync.dma_start(
                                            wgvt[:, ti, :],
                                            wd["WgvT"].ap()[
                                                t * P:(t + 1) * P,
                                                n * S:(n + 1) * S])
                                    for m in range(4):
                                        for ti in range(TC):
                                            t = ci * TC + ti
                                            nc.tensor.matmul(
                                                vmain[m][:],
                                                h1[:, t, m * P:(m + 1) * P],
                                                wvt[:, ti, :],
                                                start=(t == 0), stop=False)
                                        for ti in range(TC):
                                            t = ci * TC + ti
                                            nc.tensor.matmul(
                                                vgate[m][:],
                                                h1[:, t, m * P:(m + 1) * P],
                                                wgvt[:, ti, :],
                                                start=(t == 0), stop=False)
                                for m in range(4):
                                    nc.tensor.matmul(
                                        vmain[m][:], ones_row[:],
                                        bvrow[:, n * S:(n + 1) * S],
                                        start=False, stop=True)
                                    nc.tensor.matmul(
                                        vgate[m][:], ones_row[:],
                                        bgvrow[:, n * S:(n + 1) * S],
                                        start=False, stop=True)
                                    vsig = wvp.tile([P, S], bf16,
                                                    tag="vsig", bufs=3,
                                                    name=f"vsig_{n}_{m}")
                                    nc.scalar.activation(vsig[:],
                                                         vgate[m][:],
                                                         AF.Sigmoid)
                                    vout = wvp.tile([P, S], bf16,
                                                    tag="vout", bufs=3,
                                                    name=f"vout_{n}_{m}")
                                    nc.vector.tensor_tensor(
                                        vout[:], vmain[m][:], vsig[:],
                                        op=ALU.mult)
                                    nc.scalar.dma_start(
                                        vN_bounce[m * P:(m + 1) * P,
                                                  n * S:(n + 1) * S],
                                        vout[:])

                        nc.gpsimd.collective_compute(
                            "AllGather", ALU.bypass, ins=[vN_bounce[:]],
                            outs=[vgN[:]], replica_groups=RG)

                        # ---- K projection (T-layout) + AllGather ----
                        with (
                            tc.tile_pool(name="wproj", bufs=1) as wpool,
                            tc.tile_pool(name="pjps", bufs=1,
                                         space="PSUM") as pjps,
                        ):
                            def k_epi(j, main, sig):
                                kv = wpool.tile([P, S], bf16, tag="kv_out",
                                                name=f"kv_k_{j}", bufs=3)
                                nc.vector.scalar_tensor_tensor(
                                    kv[:], main[:], bias["bk"][:, j:j + 1],
                                    sig[:], op0=ALU.add, op1=ALU.mult)
                                nc.scalar.dma_start(
                                    k_bounce[j * P:(j + 1) * P, :], kv[:])

                            proj_gated(h1, NT, NT, "Wk", "Wgk", "bgk",
                                       wpool, pjps, k_epi)

                            nc.gpsimd.collective_compute(
                                "AllGather", ALU.bypass, ins=[k_bounce[:]],
                                outs=[kg[:]], replica_groups=RG)

                            def q_epi(j, main, sig):
                                nc.vector.scalar_tensor_tensor(
                                    q[:, j, :], main[:],
                                    bias["bq"][:, j:j + 1],
                                    sig[:], op0=ALU.add, op1=ALU.mult)

                            proj_gated(h1, NT, NT, "Wq", "Wgq", "bgq",
                                       wpool, pjps, q_epi)

                    # ---- phase B: attention ----
                    with (
                        tc.tile_pool(name="vres", bufs=1) as vresp,
                        tc.tile_pool(name="kstream", bufs=2) as kpool,
                        tc.tile_pool(name="apool", bufs=4) as apool,
                        tc.tile_pool(name="atps", bufs=1,
                                     space="PSUM") as atps,
                    ):
                        y = ypool.tile([P, NT, S], bf16)
                        # V resident [k-part, kb, d]; plain loads from the
                        # gathered N-layout V, issued on the GpSimd queue.
                        Vt = vresp.tile([P, NKB, D], bf16)
                        for kb in range(NKB):
                            nc.gpsimd.dma_start(
                                Vt[:, kb, :],
                                vgN[kb * P:(kb + 1) * P, :])

                        head_state = {}

                        def finalize_head(h, Zp_h, Yp_h):
                            urow = rows.tile([1, S], fp32, name=f"u_{h}",
                                             tag="urow", bufs=2)
                            nc.vector.reciprocal(urow[:], Zp_h[:])
                            ubf = rows.tile([1, S], bf16, name=f"ubf_{h}",
                                            tag="ubf", bufs=2)
                            nc.vector.tensor_copy(ubf[:], urow[:])
                            Up = atps.tile([P, S], fp32, name=f"Up_{h}",
                                           tag="logits", bufs=4)
                            nc.tensor.matmul(Up[:], ones_row[:], ubf[:])
                            Us = apool.tile([P, S], bf16, tag="Us",
                                            name=f"Us_{h}")
                            nc.vector.tensor_copy(Us[:], Up[:])
                            nc.vector.tensor_tensor(y[:, h, :], Yp_h[:],
                                                    Us[:], op=ALU.mult)

                        for hh in range(H):
                            Kh = kpool.tile([P, NKB * P], bf16, tag="Kh",
                                            name=f"Kh_{hh}")
                            for s_ in range(GROUP):
                                nc.gpsimd.dma_start(
                                    Kh[:, s_ * S:(s_ + 1) * S],
                                    kg[s_ * D + hh * P:
                                       s_ * D + (hh + 1) * P, :])
                            Zp = atps.tile([1, S], fp32, name=f"Z_{hh}",
                                           tag="Zp", bufs=2)
                            Yp = atps.tile([P, S], fp32, name=f"Y_{hh}",
                                           tag="Yp", bufs=2)
                            ats = {}

                            def do_L(kb, hh=hh, Kh=Kh, ats=ats):
                                Lp = atps.tile([P, S], fp32,
                                               name=f"L_{hh}_{kb}",
                                               tag="logits", bufs=4)
                                nc.tensor.matmul(
                                    Lp[:], Kh[:, kb * P:(kb + 1) * P],
                                    q[:, hh, :])
                                At = apool.tile([P, S], bf16, tag="At",
                                                name=f"At_{hh}_{kb}",
                                                bufs=6)
                                nc.scalar.activation(At[:], Lp[:], AF.Exp,
                                                     scale=ISCALE)
                                ats[kb] = At

                            do_L(0)
                            do_L(1)
                            for kb in range(NKB):
                                if kb + 2 < NKB:
                                    do_L(kb + 2)
                                nc.tensor.matmul(Zp[:], ones_col[:],
                                                 ats[kb][:],
                                                 start=(kb == 0),
                                                 stop=(kb == NKB - 1))
                                nc.tensor.matmul(
                                    Yp[:],
                                    Vt[:, kb, hh * P:(hh + 1) * P],
                                    ats[kb][:],
                                    start=(kb == 0),
                                    stop=(kb == NKB - 1))
                                if kb == 3 and hh > 0:
                                    finalize_head(hh - 1,
                                                  *head_state[hh - 1])
                            head_state[hh] = (Zp, Yp)
                        finalize_head(H - 1, *head_state[H - 1])

                  # ---- phase C: o-proj + residual ----
                  x2 = x2p.tile([P, NT, S], fp32, name="x2")
                  with (
                      tc.tile_pool(name="wproj2", bufs=1) as wpool2,
                      tc.tile_pool(name="pj2ps", bufs=1,
                                   space="PSUM") as pj2ps,
                  ):
                      def o_epi(j, main, sig):
                          tmp = wpool2.tile([P, S], fp32, tag="o_tmp",
                                            name=f"o_tmp_{j}", bufs=3)
                          nc.vector.scalar_tensor_tensor(
                              tmp[:], main[:], bias["bo"][:, j:j + 1],
                              sig[:], op0=ALU.add, op1=ALU.mult)
                          nc.vector.tensor_tensor(x2[:, j, :], tmp[:],
                                                  xt[:, j, :],
                                                  op=ALU.add)

                      proj_gated(y, NT, NT, "Wo", "Wgo", "bgo",
                                 wpool2, pj2ps, o_epi)

              # ---- phase D: LN2 + MLP ----
              with tc.tile_pool(name="midp", bufs=1) as midp:
                  mid = midp.tile([P, NF, S], bf16)
                  with tc.tile_pool(name="h2p", bufs=1) as h2p:
                      with (
                          tc.tile_pool(name="ln2tmp", bufs=1) as ln2tmp,
                          tc.tile_pool(name="ln2ps", bufs=1,
                                       space="PSUM") as ln2ps,
                      ):
                          h2 = ln_T(x2, "g2", "bt2", h2p, ln2tmp, ln2ps,
                                    "ln2")

                      with (
                          tc.tile_pool(name="wmlp1", bufs=1) as wm1,
                          tc.tile_pool(name="m1ps", bufs=1,
                                       space="PSUM") as m1ps,
                      ):
                          def mid_epi(j, main, sig):
                              tmp = wm1.tile([P, S], fp32, tag="mid_tmp",
                                             name=f"mid_tmp_{j}", bufs=3)
                              nc.vector.scalar_tensor_tensor(
                                  tmp[:], main[:],
                                  bias["bin"][:, j:j + 1], sig[:],
                                  op0=ALU.add, op1=ALU.mult)
                              nc.scalar.activation(mid[:, j, :], tmp[:],
                                                   AF.Gelu)

                          proj_gated(h2, NT, NF, "Win", "Wgin", "bgin",
                                     wm1, m1ps, mid_epi)

                  with (
                      tc.tile_pool(name="wmlp2", bufs=1) as wm2,
                      tc.tile_pool(name="m2ps", bufs=1,
                                   space="PSUM") as m2ps,
                  ):
                      def out_epi(j, main, sig):
                          tmp = wm2.tile([P, S], fp32, tag="out_tmp",
                                         name=f"out_tmp_{j}", bufs=3)
                          nc.vector.scalar_tensor_tensor(
                              tmp[:], main[:], bias["bout"][:, j:j + 1],
                              sig[:], op0=ALU.add, op1=ALU.mult)
                          outf = wm2.tile([P, S], fp32, tag="out_f",
                                          name=f"out_f_{j}", bufs=3)
                          nc.vector.tensor_tensor(outf[:], tmp[:],
                                                  x2[:, j, :], op=ALU.add)
                          nc.sync.dma_start(
                              out_d.ap()[j * P:(j + 1) * P, :], outf[:])

                      proj_gated(mid, NF, NT, "Wout", "Wgout", "bgout",
                                 wm2, m2ps, out_epi, tchunk=32, wbufs=2)

    nc.compile()
    return nc


def _prep_shared_inputs(inputs):
    m = {}
    for nm, w in (("Wq", "W_q"), ("Wgq", "Wg_q"), ("Wk", "W_k"),
                  ("Wgk", "Wg_k"), ("Wo", "W_o"), ("Wgo", "Wg_o"),
                  ("Win", "W_in"), ("Wgin", "Wg_in"), ("Wout", "W_out"),
                  ("Wgout", "Wg_out")):
        m[nm] = _w_tiled(np.asarray(inputs[w]))
    m["WvT"] = np.ascontiguousarray(np.asarray(inputs["W_v"]).T.astype(_BF))
    m["WgvT"] = np.ascontiguousarray(np.asarray(inputs["Wg_v"]).T.astype(_BF))
    m["bvrow"] = np.asarray(inputs["b_v"]).astype(_BF).reshape(1, D)
    m["bgvrow"] = np.asarray(inputs["bg_v"]).astype(_BF).reshape(1, D)
    for nm, bn in (("bq", "b_q"), ("bgq", "bg_q"), ("bk", "b_k"),
                   ("bgk", "bg_k"), ("bo", "b_o"), ("bgo", "bg_o"),
                   ("bin", "b_in"), ("bgin", "bg_in"), ("bout", "b_out"),
                   ("bgout", "bg_out"), ("g1", "ln1_g"), ("bt1", "ln1_b"),
                   ("g2", "ln2_g"), ("bt2", "ln2_b")):
        m[nm] = _b_cols(np.asarray(inputs[bn]))
    return m


def _install_trace_shim():
    """Provide antenv.axon_hooks (NTFF profiling) if the image lacks it."""
    import contextlib
    import ctypes
    import types

    try:
        import antenv.axon_hooks  # noqa: F401
        return
    except ImportError:
        pass
    try:
        import antenv
    except ImportError:
        return
    so_path = "/opt/axon/libaxon_pjrt.so"
    try:
        lib = ctypes.CDLL(so_path)
    except OSError:
        return
    if not hasattr(lib, "axon_start_nrt_profile"):
        return
    lib.axon_start_nrt_profile.argtypes = [ctypes.POINTER(ctypes.c_int64),
                                           ctypes.c_size_t]
    lib.axon_start_nrt_profile.restype = ctypes.c_int64
    lib.axon_stop_nrt_profile.argtypes = [ctypes.c_char_p]
    lib.axon_stop_nrt_profile.restype = ctypes.c_int64

    @contextlib.contextmanager
    def hook(output_dir, device_ids):
        import jax

        jax.devices()
        if device_ids:
            ids = (ctypes.c_int64 * len(device_ids))(*device_ids)
            rc = lib.axon_start_nrt_profile(ids, len(device_ids))
        else:
            rc = lib.axon_start_nrt_profile(None, 0)
        if rc != 0:
            raise RuntimeError(f"axon_start_nrt_profile rc={rc}")
        try:
            yield
        finally:
            n = lib.axon_stop_nrt_profile(str(output_dir).encode())
            print(f"profile: {n} ntff file(s) in {output_dir}",
                  file=sys.stderr)

    mod = types.ModuleType("antenv.axon_hooks")
    mod.get_axon_ntff_profile_hook = lambda: hook
    mod.set_axon_ntff_profile_hook = lambda h: None
    sys.modules["antenv.axon_hooks"] = mod
    antenv.axon_hooks = mod


LAST_RESULTS = None


def kernel(_trace=False, **inputs):
    global _COMPILED, LAST_RESULTS
    from concourse import bass_utils

    if _trace:
        _install_trace_shim()

    if _COMPILED is None:
        _COMPILED = _build()
    nc = _COMPILED

    shared = _prep_shared_inputs(inputs)
    x = np.asarray(inputs["x"], dtype=np.float32)  # [B, T, D]
    in_maps = []
    for c in range(N_CORES):
        g, s = divmod(c, GROUP)
        xT_c = np.ascontiguousarray(x[g, s * S:(s + 1) * S, :].T)
        m = dict(shared)
        m["xT"] = xT_c
        in_maps.append(m)

    LAST_RESULTS = bass_utils.run_bass_kernel_spmd(
        nc, in_maps, core_ids=list(range(N_CORES)), trace=_trace)

    out = np.empty((B, T, D), dtype=np.float32)
    for c in range(N_CORES):
        g, s = divmod(c, GROUP)
        out[g, s * S:(s + 1) * S, :] = LAST_RESULTS.results[c]["outT"].T
    return out



# revision 4
# speedup vs baseline: 1.4114x; 1.4114x over previous
"""Trainium2 Bass kernel for nn_DGEBlock (dense transformer block with
MoE-gated linears), distributed over 8 NeuronCores.

Sharding: data-parallel over batch (2 groups of 4 cores) x sequence-parallel
over tokens within each batch (512 tokens per core). Weights are replicated.
Activations live feature-major ("T-layout": [d, tok]) in SBUF so projections
are lhsT=W^T-tile @ rhs=activation with no activation transposes. V is
projected in token-major (N-)layout directly so attention's PV matmuls need
no transposes either.

Precision scheme (fp8 DoubleRow halves the matmul count where used):
  - q/k/v/o projections: main+gate both fp8e4 DoubleRow (weights stored
    64x in e4m3; epilogues fold the 1/64 into activation scales).
  - MLP in/out: MAIN path stays bf16 (accuracy: errors there land on the
    residual stream through the widest matrices); GATE path fp8 DoubleRow
    (sigmoid compresses quantization noise).
  - Attention: At (exp logits) and V stored fp8; PV and the softmax
    denominator (Z) matmuls run DoubleRow over key-block pairs; QK stays
    bf16 (contraction is only 128 so DoubleRow can't help).
  - LayerNorm stats summed from fp8 copies via DoubleRow ones-matmuls
    (2048-way averaging makes this noise negligible).
The only collectives are two 4-rank AllGathers (V in fp8, K in bf16).
Output is returned token-sharded and reassembled on host.
"""

import sys

for _p in ("/opt/trn_rl_repo",):
    if _p not in sys.path:
        sys.path.append(_p)

import numpy as np
import ml_dtypes

# ---------------------------------------------------------------- constants
B = 2
T = 2048
D = 2048
H = 16
HD = 128
FF = 4 * D  # 8192
EPS = 1e-5

N_CORES = 8
GROUP = 4  # cores per batch group (sequence-parallel degree)
S = T // GROUP  # tokens per core = 512
P = 128
NT = D // P  # 16 feature tiles
NF = FF // P  # 64 hidden tiles
NKB = T // P  # 16 key blocks per batch
NPAIR = NKB // 2
ISCALE = 1.0 / float(np.sqrt(HD))

WS = 64.0  # fp8 weight pre-scale (keeps 0.02-std weights out of subnormals)
INV = 1.0 / WS
EXPSCALE = ISCALE / (WS * WS)  # q and k are both stored at 64x

RG = [[0, 1, 2, 3], [4, 5, 6, 7]]

_BF = ml_dtypes.bfloat16
_F8 = ml_dtypes.float8_e4m3

_COMPILED = None


# ------------------------------------------------------------- host prep
def _w_tiled(W, scale, dt):
    """W [dout, din] -> [nj, 128, nt, 128] such that
    out[j, p, t, jc] == scale*W[j*128+jc, t*128+p]  (= W^T tile (t, j))."""
    dout, din = W.shape
    nj, nt = dout // P, din // P
    return np.ascontiguousarray(
        (W.reshape(nj, P, nt, P) * scale).transpose(0, 3, 2, 1).astype(dt)
    )


def _b_cols(b, scale=1.0):
    """b [dout] -> [128, nj] fp32: column j holds scale*b[j*128:(j+1)*128]."""
    nj = b.shape[0] // P
    return np.ascontiguousarray((b * scale).reshape(nj, P).T.astype(np.float32))


# ------------------------------------------------------------- device build
def _build():
    from concourse import bacc, tile, mybir

    fp32 = mybir.dt.float32
    bf16 = mybir.dt.bfloat16
    f8 = mybir.dt.float8e4
    AF = mybir.ActivationFunctionType
    ALU = mybir.AluOpType
    DR = mybir.MatmulPerfMode.DoubleRow

    nc = bacc.Bacc("TRN2", target_bir_lowering=False, debug=False,
                   num_devices=N_CORES)

    # ---- I/O tensors
    xT_d = nc.dram_tensor("xT", [D, S], fp32, kind="ExternalInput")
    wd = {}
    for nm in ("Wq", "Wgq", "Wk", "Wgk", "Wo", "Wgo"):
        wd[nm] = nc.dram_tensor(nm, [NT, P, NT, P], f8, kind="ExternalInput")
    wd["Win"] = nc.dram_tensor("Win", [NF, P, NT, P], bf16,
                               kind="ExternalInput")
    wd["Wgin"] = nc.dram_tensor("Wgin", [NF, P, NT, P], f8,
                                kind="ExternalInput")
    wd["Wout"] = nc.dram_tensor("Wout", [NT, P, NF, P], bf16,
                                kind="ExternalInput")
    wd["Wgout"] = nc.dram_tensor("Wgout", [NT, P, NF, P], f8,
                                 kind="ExternalInput")
    # V projection runs in N-layout: plain W^T [din, dout] + bias rows
    wd["WvT"] = nc.dram_tensor("WvT", [D, D], f8, kind="ExternalInput")
    wd["WgvT"] = nc.dram_tensor("WgvT", [D, D], f8, kind="ExternalInput")
    bvrow_d = nc.dram_tensor("bvrow", [1, D], bf16, kind="ExternalInput")
    bgvrow_d = nc.dram_tensor("bgvrow", [1, D], bf16, kind="ExternalInput")
    bd = {}
    for nm in ("bq", "bgq", "bk", "bgk", "bo", "bgo",
               "bout", "bgout", "g1", "bt1", "g2", "bt2"):
        bd[nm] = nc.dram_tensor(nm, [P, NT], fp32, kind="ExternalInput")
    for nm in ("bin", "bgin"):
        bd[nm] = nc.dram_tensor(nm, [P, NF], fp32, kind="ExternalInput")
    out_d = nc.dram_tensor("outT", [D, S], fp32, kind="ExternalOutput")

    with tile.TileContext(nc) as tc:
        with (
            tc.tile_pool(name="const", bufs=1) as constp,
            tc.tile_pool(name="bias", bufs=1) as biasp,
            tc.tile_pool(name="rows", bufs=1) as rows,
            tc.tile_pool(name="dram", bufs=1, space="DRAM") as dramp,
        ):
            ones_col = constp.tile([P, 1], bf16)
            nc.vector.memset(ones_col[:], 1.0)
            ones_row = constp.tile([1, P], bf16)
            nc.vector.memset(ones_row[:], 1.0)
            # fp8 "ones" pair for DoubleRow contractions with unit weights;
            # [P, 2, 16] so the pair-step is 16B (DoubleRow AP constraint)
            ones2 = constp.tile([P, 2, 16], f8)
            nc.vector.memset(ones2[:], 1.0)
            eps_t = constp.tile([1, 1], fp32)
            nc.vector.memset(eps_t[:], EPS)
            bvrow = constp.tile([1, D], bf16)
            nc.sync.dma_start(bvrow[:], bvrow_d.ap())
            bgvrow = constp.tile([1, D], bf16)
            nc.sync.dma_start(bgvrow[:], bgvrow_d.ap())

            bias = {}
            for nm in bd:
                ncols = NF if nm in ("bin", "bgin") else NT
                btile = biasp.tile([P, ncols], fp32, name=f"bias_{nm}")
                nc.sync.dma_start(btile[:], bd[nm].ap())
                bias[nm] = btile

            # ---------- helpers ----------
            def ln_T(src, gname, bname, outs, tmpool, psln, name):
                """LayerNorm over the feature dim of a T-layout activation.

                src: SBUF tile [128, NT, S] fp32.  outs: list of
                (pool, dtype) -> returns one [128, NT, S] tile per entry.
                Stats via fp8 DoubleRow ones-matmuls (contract over
                partitions); per-token scale/shift rows are broadcast to
                [128, S] via rank-1 matmuls.
                """
                S1 = psln.tile([1, S], fp32, name=f"{name}_S1", tag="ln_S1")
                S2 = psln.tile([1, S], fp32, name=f"{name}_S2", tag="ln_S2")
                for pi in range(NT // 2):
                    xp = tmpool.tile([P, 2, S], f8, tag="ln_x8", bufs=2,
                                     name=f"{name}_x8_{pi}")
                    sp = tmpool.tile([P, 2, S], f8, tag="ln_sq", bufs=2,
                                     name=f"{name}_sq_{pi}")
                    for i in range(2):
                        t = 2 * pi + i
                        nc.vector.tensor_copy(xp[:, i, :], src[:, t, :])
                        nc.scalar.activation(sp[:, i, :], src[:, t, :],
                                             AF.Square)
                    nc.tensor.matmul(S1[:], ones2[:, :, 0:1], xp[:],
                                     start=(pi == 0),
                                     stop=(pi == NT // 2 - 1), perf_mode=DR)
                    nc.tensor.matmul(S2[:], ones2[:, :, 0:1], sp[:],
                                     start=(pi == 0),
                                     stop=(pi == NT // 2 - 1), perf_mode=DR)

                def row(nm, dt=fp32):
                    return rows.tile([1, S], dt, name=f"{name}_{nm}",
                                     tag=f"ln_{nm}")

                mean = row("mean")
                nc.vector.tensor_scalar_mul(mean[:], S1[:], 1.0 / D)
                m2 = row("m2")
                nc.vector.tensor_scalar_mul(m2[:], S2[:], 1.0 / D)
                msq = row("msq")
                nc.vector.tensor_tensor(msq[:], mean[:], mean[:],
                                        op=ALU.mult)
                var = row("var")
                nc.vector.tensor_tensor(var[:], m2[:], msq[:],
                                        op=ALU.subtract)
                std = row("std")
                nc.scalar.activation(std[:], var[:], AF.Sqrt,
                                     bias=eps_t[:])
                rstd = row("rstd")
                nc.vector.reciprocal(rstd[:], std[:])
                rstd_bf = row("rstdbf", bf16)
                nc.vector.tensor_copy(rstd_bf[:], rstd[:])
                mr_bf = row("mrbf", bf16)
                nc.vector.tensor_tensor(mr_bf[:], mean[:], rstd[:],
                                        op=ALU.mult)
                Ab_p = psln.tile([P, S], fp32, name=f"{name}_Abp",
                                 tag="ln_Abp")
                nc.tensor.matmul(Ab_p[:], ones_row[:], rstd_bf[:])
                Bb_p = psln.tile([P, S], fp32, name=f"{name}_Bbp",
                                 tag="ln_Bbp")
                nc.tensor.matmul(Bb_p[:], ones_row[:], mr_bf[:])
                Ab = tmpool.tile([P, S], fp32, name=f"{name}_Ab")
                nc.vector.tensor_copy(Ab[:], Ab_p[:])
                Bb = tmpool.tile([P, S], fp32, name=f"{name}_Bb")
                nc.vector.tensor_copy(Bb[:], Bb_p[:])
                hs = [pool.tile([P, NT, S], dt, name=f"{name}_h{i}")
                      for i, (pool, dt) in enumerate(outs)]
                for t in range(NT):
                    tmp = tmpool.tile([P, S], fp32, name=f"{name}_t0_{t}",
                                      tag="ln_t0", bufs=3)
                    nc.vector.tensor_tensor(tmp[:], src[:, t, :], Ab[:],
                                            op=ALU.mult)
                    tmp2 = tmpool.tile([P, S], fp32, name=f"{name}_t1_{t}",
                                       tag="ln_t1", bufs=3)
                    nc.vector.tensor_tensor(tmp2[:], tmp[:], Bb[:],
                                            op=ALU.subtract)
                    for h in hs:
                        nc.scalar.activation(h[:, t, :], tmp2[:], AF.Identity,
                                             bias=bias[bname][:, t:t + 1],
                                             scale=bias[gname][:, t:t + 1])
                return hs

            def proj_gated(nt, nj, main_spec, gate_spec, bgname, wpool,
                           pspool, epilogue, tchunk=None, wbufs=3):
                """Gated projection in T-layout.  spec = (wname, src, dr,
                wdtype).  dr=True runs fp8 DoubleRow over k-tile pairs."""
                if tchunk is None:
                    tchunk = nt
                nchunk = nt // tchunk
                wname, src_m, dr_m, dt_m = main_spec
                wgname, src_g, dr_g, dt_g = gate_spec
                for j in range(nj):
                    main = pspool.tile([P, S], fp32, name=f"{wname}_m{j}",
                                       tag="pj_main", bufs=2)
                    gate = pspool.tile([P, S], fp32, name=f"{wname}_g{j}",
                                       tag="pj_gate", bufs=2)

                    def path(acc, wnm, src, dr, wdt, tag):
                        for ci in range(nchunk):
                            wt = wpool.tile([P, tchunk, P], wdt, tag=tag,
                                            name=f"w_{wnm}_{j}_{ci}",
                                            bufs=wbufs)
                            nc.sync.dma_start(
                                wt[:],
                                wd[wnm].ap()[j, :,
                                             ci * tchunk:(ci + 1) * tchunk,
                                             :])
                            if dr:
                                for pi in range(tchunk // 2):
                                    t = ci * tchunk + 2 * pi
                                    nc.tensor.matmul(
                                        acc[:], wt[:, 2 * pi:2 * pi + 2, :],
                                        src[:, t:t + 2, :],
                                        start=(t == 0), stop=(t == nt - 2),
                                        perf_mode=DR)
                            else:
                                for ti in range(tchunk):
                                    t = ci * tchunk + ti
                                    nc.tensor.matmul(
                                        acc[:], wt[:, ti, :], src[:, t, :],
                                        start=(t == 0), stop=(t == nt - 1))

                    path(main, wname, src_m, dr_m, dt_m, "wmain")
                    path(gate, wgname, src_g, dr_g, dt_g, "wgate")
                    sig = wpool.tile([P, S], bf16, tag="sig",
                                     name=f"sig_{wname}_{j}", bufs=3)
                    nc.scalar.activation(sig[:], gate[:], AF.Sigmoid,
                                         bias=bias[bgname][:, j:j + 1],
                                         scale=(INV if dr_g else 1.0))
                    epilogue(j, main, sig)

            # x2 outlives phases A-C (used by LN2 + MLP residual)
            with tc.tile_pool(name="x2p", bufs=1) as x2p:
              with tc.tile_pool(name="xt", bufs=1) as xtp:
                xt = xtp.tile([P, NT, S], fp32)
                xT_v = xT_d.ap().rearrange("(t p) s -> t p s", p=P)
                for t in range(NT):
                    nc.sync.dma_start(xt[:, t, :], xT_v[t])

                vN_bounce = dramp.tile([S, D], f8)
                k_bounce = dramp.tile([D, S], bf16)
                vgN = dramp.tile([GROUP * S, D], f8)
                kg = dramp.tile([GROUP * D, S], bf16)

                with tc.tile_pool(name="yp", bufs=1) as ypool:
                  with tc.tile_pool(name="qp", bufs=1) as qpool:
                    q = qpool.tile([P, NT, S], bf16)

                    with tc.tile_pool(name="hq", bufs=1) as hqp:
                        with (
                            tc.tile_pool(name="ln1tmp", bufs=1) as ln1tmp,
                            tc.tile_pool(name="ln1ps", bufs=1,
                                         space="PSUM") as ln1ps,
                        ):
                            (h1,) = ln_T(xt, "g1", "bt1", [(hqp, f8)],
                                         ln1tmp, ln1ps, "ln1")

                        # ---- V projection, N-layout, fp8 DoubleRow ----
                        with (
                            tc.tile_pool(name="wv", bufs=1) as wvp,
                            tc.tile_pool(name="vps", bufs=1,
                                         space="PSUM") as vps,
                        ):
                            TC = NT // 2
                            for n in range(4):
                                vmain = [vps.tile([P, S], fp32,
                                                  tag="v_main", bufs=4,
                                                  name=f"vm_{n}_{m}")
                                         for m in range(4)]
                                vgate = [vps.tile([P, S], fp32,
                                                  tag="v_gate", bufs=4,
                                                  name=f"vg_{n}_{m}")
                                         for m in range(4)]
                                for ci in range(2):
                                    wvt = wvp.tile([P, TC, 4 * P], f8,
                                                   tag="wv", bufs=2,
                                                   name=f"wv_{n}_{ci}")
                                    wgvt = wvp.tile([P, TC, 4 * P], f8,
                                                    tag="wgv", bufs=2,
                                                    name=f"wgv_{n}_{ci}")
                                    for ti in range(TC):
                                        t = ci * TC + ti
                                        nc.sync.dma_start(
                                            wvt[:, ti, :],
                                            wd["WvT"].ap()[t * P:(t + 1) * P,
                                                           n * S:(n + 1) * S])
                                        nc.sync.dma_start(
                                            wgvt[:, ti, :],
                                            wd["WgvT"].ap()[
                                                t * P:(t + 1) * P,
                                                n * S:(n + 1) * S])
                                    for m in range(4):
                                        for pi in range(TC // 2):
                                            t = ci * TC + 2 * pi
                                            nc.tensor.matmul(
                                                vmain[m][:],
                                                h1[:, t:t + 2,
                                                   m * P:(m + 1) * P],
                                                wvt[:, 2 * pi:2 * pi + 2, :],
                                                start=(t == 0), stop=False,
                                                perf_mode=DR)
                                        for pi in range(TC // 2):
                                            t = ci * TC + 2 * pi
                                            nc.tensor.matmul(
                                                vgate[m][:],
                                                h1[:, t:t + 2,
                                                   m * P:(m + 1) * P],
                                                wgvt[:, 2 * pi:2 * pi + 2, :],
                                                start=(t == 0), stop=False,
                                                perf_mode=DR)
                                for m in range(4):
                                    nc.tensor.matmul(
                                        vmain[m][:], ones_row[:],
                                        bvrow[:, n * S:(n + 1) * S],
                                        start=False, stop=True)
                                    nc.tensor.matmul(
                                        vgate[m][:], ones_row[:],
                                        bgvrow[:, n * S:(n + 1) * S],
                                        start=False, stop=True)
                                    vsig = wvp.tile([P, S], bf16,
                                                    tag="vsig", bufs=3,
                                                    name=f"vsig_{n}_{m}")
                                    nc.scalar.activation(vsig[:],
                                                         vgate[m][:],
                                                         AF.Sigmoid,
                                                         scale=INV)
                                    vout = wvp.tile([P, S], f8,
                                                    tag="vout", bufs=3,
                                                    name=f"vout_{n}_{m}")
                                    nc.vector.scalar_tensor_tensor(
                                        vout[:], vmain[m][:], INV, vsig[:],
                                        op0=ALU.mult, op1=ALU.mult)
                                    nc.scalar.dma_start(
                                        vN_bounce[m * P:(m + 1) * P,
                                                  n * S:(n + 1) * S],
                                        vout[:])

                        nc.gpsimd.collective_compute(
                            "AllGather", ALU.bypass, ins=[vN_bounce[:]],
                            outs=[vgN[:]], replica_groups=RG)

                        # ---- K projection (T-layout) + AllGather ----
                        with (
                            tc.tile_pool(name="wproj", bufs=1) as wpool,
                            tc.tile_pool(name="pjps", bufs=1,
                                         space="PSUM") as pjps,
                        ):
                            def k_epi(j, main, sig):
                                kv = wpool.tile([P, S], bf16, tag="kv_out",
                                                name=f"kv_k_{j}", bufs=3)
                                nc.vector.scalar_tensor_tensor(
                                    kv[:], main[:], bias["bk"][:, j:j + 1],
                                    sig[:], op0=ALU.add, op1=ALU.mult)
                                nc.scalar.dma_start(
                                    k_bounce[j * P:(j + 1) * P, :], kv[:])

                            proj_gated(NT, NT, ("Wk", h1, True, f8),
                                       ("Wgk", h1, True, f8), "bgk",
                                       wpool, pjps, k_epi)

                            nc.gpsimd.collective_compute(
                                "AllGather", ALU.bypass, ins=[k_bounce[:]],
                                outs=[kg[:]], replica_groups=RG)

                            def q_epi(j, main, sig):
                                nc.vector.scalar_tensor_tensor(
                                    q[:, j, :], main[:],
                                    bias["bq"][:, j:j + 1],
                                    sig[:], op0=ALU.add, op1=ALU.mult)

                            proj_gated(NT, NT, ("Wq", h1, True, f8),
                                       ("Wgq", h1, True, f8), "bgq",
                                       wpool, pjps, q_epi)

                    # ---- phase B: attention ----
                    with (
                        tc.tile_pool(name="vres", bufs=1) as vresp,
                        tc.tile_pool(name="kstream", bufs=2) as kpool,
                        tc.tile_pool(name="apool", bufs=4) as apool,
                        tc.tile_pool(name="atps", bufs=1,
                                     space="PSUM") as atps,
                    ):
                        y = ypool.tile([P, NT, S], f8)
                        # V resident [k-part, kb, d] fp8; plain loads from
                        # the gathered N-layout V, on the GpSimd queue.
                        Vt = vresp.tile([P, NKB, D], f8)
                        for kb in range(NKB):
                            nc.gpsimd.dma_start(
                                Vt[:, kb, :],
                                vgN[kb * P:(kb + 1) * P, :])

                        head_state = {}

                        def finalize_head(h, Zp_h, Yp_h):
                            urow = rows.tile([1, S], fp32, name=f"u_{h}",
                                             tag="urow", bufs=2)
                            nc.vector.reciprocal(urow[:], Zp_h[:])
                            ubf = rows.tile([1, S], bf16, name=f"ubf_{h}",
                                            tag="ubf", bufs=2)
                            nc.vector.tensor_copy(ubf[:], urow[:])
                            Up = atps.tile([P, S], fp32, name=f"Up_{h}",
                                           tag="logits", bufs=4)
                            nc.tensor.matmul(Up[:], ones_row[:], ubf[:])
                            Us = apool.tile([P, S], bf16, tag="Us",
                                            name=f"Us_{h}")
                            nc.vector.tensor_copy(Us[:], Up[:])
                            nc.vector.tensor_tensor(y[:, h, :], Yp_h[:],
                                                    Us[:], op=ALU.mult)

                        for hh in range(H):
                            Kh = kpool.tile([P, NKB * P], bf16, tag="Kh",
                                            name=f"Kh_{hh}")
                            for s_ in range(GROUP):
                                nc.gpsimd.dma_start(
                                    Kh[:, s_ * S:(s_ + 1) * S],
                                    kg[s_ * D + hh * P:
                                       s_ * D + (hh + 1) * P, :])
                            Zp = atps.tile([1, S], fp32, name=f"Z_{hh}",
                                           tag="Zp", bufs=2)
                            Yp = atps.tile([P, S], fp32, name=f"Y_{hh}",
                                           tag="Yp", bufs=2)
                            ats = {}

                            def do_pair(pi, hh=hh, Kh=Kh, ats=ats):
                                At2 = apool.tile([P, 2, S], f8, tag="At2",
                                                 name=f"At2_{hh}_{pi}",
                                                 bufs=4)
                                for i in range(2):
                                    kb = 2 * pi + i
                                    Lp = atps.tile([P, S], fp32,
                                                   name=f"L_{hh}_{kb}",
                                                   tag="logits", bufs=4)
                                    nc.tensor.matmul(
                                        Lp[:], Kh[:, kb * P:(kb + 1) * P],
                                        q[:, hh, :])
                                    nc.scalar.activation(At2[:, i, :], Lp[:],
                                                         AF.Exp,
                                                         scale=EXPSCALE)
                                ats[pi] = At2

                            do_pair(0)
                            do_pair(1)
                            for pi in range(NPAIR):
                                if pi + 2 < NPAIR:
                                    do_pair(pi + 2)
                                At2 = ats.pop(pi)
                                nc.tensor.matmul(Zp[:], ones2[:, :, 0:1],
                                                 At2[:],
                                                 start=(pi == 0),
                                                 stop=(pi == NPAIR - 1),
                                                 perf_mode=DR)
                                nc.tensor.matmul(
                                    Yp[:],
                                    Vt[:, 2 * pi:2 * pi + 2,
                                       hh * P:(hh + 1) * P],
                                    At2[:],
                                    start=(pi == 0),
                                    stop=(pi == NPAIR - 1),
                                    perf_mode=DR)
                                if pi == 2 and hh > 0:
                                    finalize_head(hh - 1,
                                                  *head_state[hh - 1])
                            head_state[hh] = (Zp, Yp)
                        finalize_head(H - 1, *head_state[H - 1])

                  # ---- phase C: o-proj + residual ----
                  x2 = x2p.tile([P, NT, S], fp32, name="x2")
                  with (
                      tc.tile_pool(name="wproj2", bufs=1) as wpool2,
                      tc.tile_pool(name="pj2ps", bufs=1,
                                   space="PSUM") as pj2ps,
                  ):
                      def o_epi(j, main, sig):
                          tmp = wpool2.tile([P, S], fp32, tag="o_tmp",
                                            name=f"o_tmp_{j}", bufs=3)
                          nc.vector.scalar_tensor_tensor(
                              tmp[:], main[:], bias["bo"][:, j:j + 1],
                              sig[:], op0=ALU.add, op1=ALU.mult)
                          nc.vector.scalar_tensor_tensor(
                              x2[:, j, :], tmp[:], INV, xt[:, j, :],
                              op0=ALU.mult, op1=ALU.add)

                      proj_gated(NT, NT, ("Wo", y, True, f8),
                                 ("Wgo", y, True, f8), "bgo",
                                 wpool2, pj2ps, o_epi)

              # ---- phase D: LN2 + MLP ----
              with tc.tile_pool(name="midp", bufs=1) as midp:
                  mid_bf = midp.tile([P, NF, S], bf16)
                  mid_f8 = midp.tile([P, NF, S], f8)
                  with tc.tile_pool(name="h2p", bufs=1) as h2p:
                      with (
                          tc.tile_pool(name="ln2tmp", bufs=1) as ln2tmp,
                          tc.tile_pool(name="ln2ps", bufs=1,
                                       space="PSUM") as ln2ps,
                      ):
                          h2_bf, h2_f8 = ln_T(x2, "g2", "bt2",
                                              [(h2p, bf16), (h2p, f8)],
                                              ln2tmp, ln2ps, "ln2")

                      with (
                          tc.tile_pool(name="wmlp1", bufs=1) as wm1,
                          tc.tile_pool(name="m1ps", bufs=1,
                                       space="PSUM") as m1ps,
                      ):
                          def mid_epi(j, main, sig):
                              tmp = wm1.tile([P, S], fp32, tag="mid_tmp",
                                             name=f"mid_tmp_{j}", bufs=3)
                              nc.vector.scalar_tensor_tensor(
                                  tmp[:], main[:],
                                  bias["bin"][:, j:j + 1], sig[:],
                                  op0=ALU.add, op1=ALU.mult)
                              nc.scalar.activation(mid_bf[:, j, :], tmp[:],
                                                   AF.Gelu)
                              nc.scalar.activation(mid_f8[:, j, :], tmp[:],
                                                   AF.Gelu)

                          proj_gated(NT, NF, ("Win", h2_bf, False, bf16),
                                     ("Wgin", h2_f8, True, f8), "bgin",
                                     wm1, m1ps, mid_epi, tchunk=8)

                  with (
                      tc.tile_pool(name="wmlp2", bufs=1) as wm2,
                      tc.tile_pool(name="m2ps", bufs=1,
                                   space="PSUM") as m2ps,
                  ):
                      def out_epi(j, main, sig):
                          tmp = wm2.tile([P, S], fp32, tag="out_tmp",
                                         name=f"out_tmp_{j}", bufs=3)
                          nc.vector.scalar_tensor_tensor(
                              tmp[:], main[:], bias["bout"][:, j:j + 1],
                              sig[:], op0=ALU.add, op1=ALU.mult)
                          outf = wm2.tile([P, S], fp32, tag="out_f",
                                          name=f"out_f_{j}", bufs=3)
                          nc.vector.tensor_tensor(outf[:], tmp[:],
                                                  x2[:, j, :], op=ALU.add)
                          nc.sync.dma_start(
                              out_d.ap()[j * P:(j + 1) * P, :], outf[:])

                      proj_gated(NF, NT, ("Wout", mid_bf, False, bf16),
                                 ("Wgout", mid_f8, True, f8), "bgout",
                                 wm2, m2ps, out_epi, tchunk=32, wbufs=2)

    nc.compile()
    return nc


def _prep_shared_inputs(inputs):
    m = {}
    # fp8 weights stored at 64x (T-layout tiles)
    for nm, w in (("Wq", "W_q"), ("Wgq", "Wg_q"), ("Wk", "W_k"),
                  ("Wgk", "Wg_k"), ("Wo", "W_o"), ("Wgo", "Wg_o"),
                  ("Wgin", "Wg_in"), ("Wgout", "Wg_out")):
        m[nm] = _w_tiled(np.asarray(inputs[w]), WS, _F8)
    # bf16 main-path MLP weights at 1x
    for nm, w in (("Win", "W_in"), ("Wout", "W_out")):
        m[nm] = _w_tiled(np.asarray(inputs[w]), 1.0, _BF)
    m["WvT"] = np.ascontiguousarray(
        (np.asarray(inputs["W_v"]).T * WS).astype(_F8))
    m["WgvT"] = np.ascontiguousarray(
        (np.asarray(inputs["Wg_v"]).T * WS).astype(_F8))
    m["bvrow"] = (np.asarray(inputs["b_v"]) * WS).astype(_BF).reshape(1, D)
    m["bgvrow"] = (np.asarray(inputs["bg_v"]) * WS).astype(_BF).reshape(1, D)
    for nm, bn, sc in (("bq", "b_q", WS), ("bgq", "bg_q", 1.0),
                       ("bk", "b_k", WS), ("bgk", "bg_k", 1.0),
                       ("bo", "b_o", WS), ("bgo", "bg_o", 1.0),
                       ("bin", "b_in", 1.0), ("bgin", "bg_in", 1.0),
                       ("bout", "b_out", 1.0), ("bgout", "bg_out", 1.0),
                       ("g1", "ln1_g", 1.0), ("bt1", "ln1_b", 1.0),
                       ("g2", "ln2_g", 1.0), ("bt2", "ln2_b", 1.0)):
        m[nm] = _b_cols(np.asarray(inputs[bn]), sc)
    return m


def _install_trace_shim():
    """Provide antenv.axon_hooks (NTFF profiling) if the image lacks it."""
    import contextlib
    import ctypes
    import types

    try:
        import antenv.axon_hooks  # noqa: F401
        return
    except ImportError:
        pass
    try:
        import antenv
    except ImportError:
        return
    so_path = "/opt/axon/libaxon_pjrt.so"
    try:
        lib = ctypes.CDLL(so_path)
    except OSError:
        return
    if not hasattr(lib, "axon_start_nrt_profile"):
        return
    lib.axon_start_nrt_profile.argtypes = [ctypes.POINTER(ctypes.c_int64),
                                           ctypes.c_size_t]
    lib.axon_start_nrt_profile.restype = ctypes.c_int64
    lib.axon_stop_nrt_profile.argtypes = [ctypes.c_char_p]
    lib.axon_stop_nrt_profile.restype = ctypes.c_int64

    @contextlib.contextmanager
    def hook(output_dir, device_ids):
        import jax

        jax.devices()
        if device_ids:
            ids = (ctypes.c_int64 * len(device_ids))(*device_ids)
            rc = lib.axon_start_nrt_profile(ids, len(device_ids))
        else:
            rc = lib.axon_start_nrt_profile(None, 0)
        if rc != 0:
            raise RuntimeError(f"axon_start_nrt_profile rc={rc}")
        try:
            yield
        finally:
            n = lib.axon_stop_nrt_profile(str(output_dir).encode())
            print(f"profile: {n} ntff file(s) in {output_dir}",
                  file=sys.stderr)

    mod = types.ModuleType("antenv.axon_hooks")
    mod.get_axon_ntff_profile_hook = lambda: hook
    mod.set_axon_ntff_profile_hook = lambda h: None
    sys.modules["antenv.axon_hooks"] = mod
    antenv.axon_hooks = mod


LAST_RESULTS = None


def kernel(_trace=False, **inputs):
    global _COMPILED, LAST_RESULTS
    from concourse import bass_utils

    if _trace:
        _install_trace_shim()

    if _COMPILED is None:
        _COMPILED = _build()
    nc = _COMPILED

    shared = _prep_shared_inputs(inputs)
    x = np.asarray(inputs["x"], dtype=np.float32)  # [B, T, D]
    in_maps = []
    for c in range(N_CORES):
        g, s = divmod(c, GROUP)
        xT_c = np.ascontiguousarray(x[g, s * S:(s + 1) * S, :].T)
        m = dict(shared)
        m["xT"] = xT_c
        in_maps.append(m)

    LAST_RESULTS = bass_utils.run_bass_kernel_spmd(
        nc, in_maps, core_ids=list(range(N_CORES)), trace=_trace)

    out = np.empty((B, T, D), dtype=np.float32)
    for c in range(N_CORES):
        g, s = divmod(c, GROUP)
        out[g, s * S:(s + 1) * S, :] = LAST_RESULTS.results[c]["outT"].T
    return out


# revision 19
# speedup vs baseline: 1.5513x; 1.0992x over previous
"""Trainium2 Bass kernel for nn_DGEBlock (dense transformer block with
MoE-gated linears), distributed over 8 NeuronCores.

Sharding: data-parallel over batch (2 groups of 4 cores) x sequence-parallel
over tokens within each batch (512 tokens per core). Weights are replicated.
Activations live feature-major ("T-layout": [d, tok]) in SBUF so projections
are lhsT=W^T-tile @ rhs=activation with no activation transposes. V is
projected in token-major (N-)layout directly so attention's PV matmuls need
no transposes either.

Precision scheme (fp8 DoubleRow halves the matmul count where used):
  - q/k/v/o projections: main+gate both fp8e4 DoubleRow (weights stored
    64x in e4m3; epilogues fold the 1/64 into activation scales).
  - MLP in/out: MAIN path stays bf16 (accuracy: errors there land on the
    residual stream through the widest matrices); GATE path fp8 DoubleRow
    (sigmoid compresses quantization noise).
  - Attention: At (exp logits) and V stored fp8; PV and the softmax
    denominator (Z) matmuls run DoubleRow over key-block pairs; QK stays
    bf16 (contraction is only 128 so DoubleRow can't help).
  - LayerNorm stats summed from fp8 copies via DoubleRow ones-matmuls
    (2048-way averaging makes this noise negligible).
The only collectives are two 4-rank AllGathers (V in fp8, K in bf16).
Output is returned token-sharded and reassembled on host.
"""

import sys

for _p in ("/opt/trn_rl_repo",):
    if _p not in sys.path:
        sys.path.append(_p)

import numpy as np
import ml_dtypes

# ---------------------------------------------------------------- constants
B = 2
T = 2048
D = 2048
H = 16
HD = 128
FF = 4 * D  # 8192
EPS = 1e-5

N_CORES = 8
GROUP = 4  # cores per batch group (sequence-parallel degree)
S = T // GROUP  # tokens per core = 512
P = 128
NT = D // P  # 16 feature tiles
NF = FF // P  # 64 hidden tiles
NKB = T // P  # 16 key blocks per batch
NPAIR = NKB // 2
ISCALE = 1.0 / float(np.sqrt(HD))

WS = 64.0  # fp8 weight pre-scale (keeps 0.02-std weights out of subnormals)
INV = 1.0 / WS
EXPSCALE = ISCALE / (WS * WS)  # q and k are both stored at 64x

RG = [[0, 1, 2, 3], [4, 5, 6, 7]]

_BF = ml_dtypes.bfloat16
_F8 = ml_dtypes.float8_e4m3

_COMPILED = None


# ------------------------------------------------------------- host prep
def _w_tiled(W, scale, dt):
    """W [dout, din] -> [nj, 128, nt, 128] such that
    out[j, p, t, jc] == scale*W[j*128+jc, t*128+p]  (= W^T tile (t, j))."""
    dout, din = W.shape
    nj, nt = dout // P, din // P
    return np.ascontiguousarray(
        (W.reshape(nj, P, nt, P) * scale).transpose(0, 3, 2, 1).astype(dt)
    )


def _b_cols(b, scale=1.0):
    """b [dout] -> [128, nj] fp32: column j holds scale*b[j*128:(j+1)*128]."""
    nj = b.shape[0] // P
    return np.ascontiguousarray((b * scale).reshape(nj, P).T.astype(np.float32))


# ------------------------------------------------------------- device build
def _build():
    from concourse import bacc, tile, mybir

    fp32 = mybir.dt.float32
    bf16 = mybir.dt.bfloat16
    f8 = mybir.dt.float8e4
    AF = mybir.ActivationFunctionType
    ALU = mybir.AluOpType
    DR = mybir.MatmulPerfMode.DoubleRow

    nc = bacc.Bacc("TRN2", target_bir_lowering=False, debug=False,
                   num_devices=N_CORES)

    # ---- I/O tensors
    xT_d = nc.dram_tensor("xT", [D, S], fp32, kind="ExternalInput")
    wd = {}
    for nm in ("Wq", "Wgq", "Wk", "Wgk", "Wo", "Wgo"):
        wd[nm] = nc.dram_tensor(nm, [NT, P, NT, P], f8, kind="ExternalInput")
    wd["Win"] = nc.dram_tensor("Win", [NF, P, NT, P], f8,
                               kind="ExternalInput")
    wd["Wgin"] = nc.dram_tensor("Wgin", [NF, P, NT, P], f8,
                                kind="ExternalInput")
    wd["Wout"] = nc.dram_tensor("Wout", [NT, P, NF, P], bf16,
                                kind="ExternalInput")
    wd["Wgout"] = nc.dram_tensor("Wgout", [NT, P, NF, P], f8,
                                 kind="ExternalInput")
    # V projection runs in N-layout: plain W^T [din, dout] + bias rows
    wd["WvT"] = nc.dram_tensor("WvT", [D, D], f8, kind="ExternalInput")
    wd["WgvT"] = nc.dram_tensor("WgvT", [D, D], f8, kind="ExternalInput")
    bvrow_d = nc.dram_tensor("bvrow", [1, D], bf16, kind="ExternalInput")
    bgvrow_d = nc.dram_tensor("bgvrow", [1, D], bf16, kind="ExternalInput")
    bd = {}
    for nm in ("bq", "bgq", "bk", "bgk", "bo", "bgo",
               "bout", "bgout", "g1", "bt1", "g2", "bt2"):
        bd[nm] = nc.dram_tensor(nm, [P, NT], fp32, kind="ExternalInput")
    for nm in ("bin", "bgin"):
        bd[nm] = nc.dram_tensor(nm, [P, NF], fp32, kind="ExternalInput")
    out_d = nc.dram_tensor("outT", [D, S], fp32, kind="ExternalOutput")

    with tile.TileContext(nc) as tc:
        with (
            tc.tile_pool(name="const", bufs=1) as constp,
            tc.tile_pool(name="bias", bufs=1) as biasp,
            tc.tile_pool(name="rows", bufs=1) as rows,
            tc.tile_pool(name="dram", bufs=1, space="DRAM") as dramp,
        ):
            ones_col = constp.tile([P, 1], bf16)
            nc.vector.memset(ones_col[:], 1.0)
            ones_row = constp.tile([1, P], bf16)
            nc.vector.memset(ones_row[:], 1.0)
            # fp8 "ones" pair for DoubleRow contractions with unit weights;
            # [P, 2, 16] so the pair-step is 16B (DoubleRow AP constraint)
            ones2 = constp.tile([P, 2, 16], f8)
            nc.vector.memset(ones2[:], 1.0)
            eps_t = constp.tile([1, 1], fp32)
            nc.vector.memset(eps_t[:], EPS)
            bvrow = constp.tile([1, D], bf16)
            nc.sync.dma_start(bvrow[:], bvrow_d.ap())
            bgvrow = constp.tile([1, D], bf16)
            nc.sync.dma_start(bgvrow[:], bgvrow_d.ap())

            bias = {}
            for nm in bd:
                ncols = NF if nm in ("bin", "bgin") else NT
                btile = biasp.tile([P, ncols], fp32, name=f"bias_{nm}")
                nc.sync.dma_start(btile[:], bd[nm].ap())
                bias[nm] = btile

            # ---------- helpers ----------
            def ln_stats_pair(S1, S2, src0, src1, tmpool, pi, npair, name):
                """Accumulate sum/sum-sq of one feature-tile pair into the
                S1/S2 psums via fp8 DoubleRow ones-matmuls."""
                xp = tmpool.tile([P, 2, S], f8, tag="ln_x8", bufs=2,
                                 name=f"{name}_x8_{pi}")
                sp = tmpool.tile([P, 2, S], f8, tag="ln_sq", bufs=2,
                                 name=f"{name}_sq_{pi}")
                for i, srct in enumerate((src0, src1)):
                    nc.vector.tensor_copy(xp[:, i, :], srct)
                    nc.scalar.activation(sp[:, i, :], srct, AF.Square)
                nc.tensor.matmul(S1[:], ones2[:, :, 0:1], xp[:],
                                 start=(pi == 0), stop=(pi == npair - 1),
                                 perf_mode=DR)
                nc.tensor.matmul(S2[:], ones2[:, :, 0:1], sp[:],
                                 start=(pi == 0), stop=(pi == npair - 1),
                                 perf_mode=DR)

            def ln_T(src, gname, bname, outs, tmpool, psln, name,
                     stats=None):
                """LayerNorm over the feature dim of a T-layout activation.

                src: SBUF tile [128, NT, S] fp32.  outs: list of
                (pool, dtype) -> returns one [128, NT, S] tile per entry.
                Stats via fp8 DoubleRow ones-matmuls (contract over
                partitions) unless passed precomputed; per-token scale/shift
                rows are broadcast to [128, S] via rank-1 matmuls.  The
                apply chain alternates Vector/GpSimd per tile to halve the
                serial latency.
                """
                if stats is None:
                    S1 = psln.tile([1, S], fp32, name=f"{name}_S1",
                                   tag="ln_S1")
                    S2 = psln.tile([1, S], fp32, name=f"{name}_S2",
                                   tag="ln_S2")
                    for pi in range(NT // 2):
                        t = 2 * pi
                        ln_stats_pair(S1, S2, src[:, t, :], src[:, t + 1, :],
                                      tmpool, pi, NT // 2, name)
                else:
                    S1, S2 = stats

                def row(nm, dt=fp32):
                    return rows.tile([1, S], dt, name=f"{name}_{nm}",
                                     tag=f"ln_{nm}")

                mean = row("mean")
                nc.vector.tensor_scalar_mul(mean[:], S1[:], 1.0 / D)
                m2 = row("m2")
                nc.vector.tensor_scalar_mul(m2[:], S2[:], 1.0 / D)
                msq = row("msq")
                nc.vector.tensor_tensor(msq[:], mean[:], mean[:],
                                        op=ALU.mult)
                var = row("var")
                nc.vector.tensor_tensor(var[:], m2[:], msq[:],
                                        op=ALU.subtract)
                std = row("std")
                nc.scalar.activation(std[:], var[:], AF.Sqrt,
                                     bias=eps_t[:])
                rstd = row("rstd")
                nc.vector.reciprocal(rstd[:], std[:])
                rstd_bf = row("rstdbf", bf16)
                nc.vector.tensor_copy(rstd_bf[:], rstd[:])
                mr_bf = row("mrbf", bf16)
                nc.vector.tensor_tensor(mr_bf[:], mean[:], rstd[:],
                                        op=ALU.mult)
                Ab_p = psln.tile([P, S], fp32, name=f"{name}_Abp",
                                 tag="ln_Abp")
                nc.tensor.matmul(Ab_p[:], ones_row[:], rstd_bf[:])
                Bb_p = psln.tile([P, S], fp32, name=f"{name}_Bbp",
                                 tag="ln_Bbp")
                nc.tensor.matmul(Bb_p[:], ones_row[:], mr_bf[:])
                Ab = tmpool.tile([P, S], fp32, name=f"{name}_Ab")
                nc.vector.tensor_copy(Ab[:], Ab_p[:])
                Bb = tmpool.tile([P, S], fp32, name=f"{name}_Bb")
                nc.vector.tensor_copy(Bb[:], Bb_p[:])
                hs = [pool.tile([P, NT, S], dt, name=f"{name}_h{i}")
                      for i, (pool, dt) in enumerate(outs)]
                for t in range(NT):
                    tmp = tmpool.tile([P, S], fp32, name=f"{name}_t0_{t}",
                                      tag="ln_t0", bufs=4)
                    nc.vector.tensor_tensor(tmp[:], src[:, t, :], Ab[:],
                                            op=ALU.mult)
                    tmp2 = tmpool.tile([P, S], fp32, name=f"{name}_t1_{t}",
                                       tag="ln_t1", bufs=4)
                    nc.vector.tensor_tensor(tmp2[:], tmp[:], Bb[:],
                                            op=ALU.subtract)
                    for h in hs:
                        nc.scalar.activation(h[:, t, :], tmp2[:], AF.Identity,
                                             bias=bias[bname][:, t:t + 1],
                                             scale=bias[gname][:, t:t + 1])
                return hs

            def proj_gated(nt, nj, main_spec, gate_spec, bgname, wpool,
                           pspool, epilogue, tchunk=None, wbufs=3):
                """Gated projection in T-layout.  spec = (wname, src, dr,
                wdtype).  dr=True runs fp8 DoubleRow over k-tile pairs."""
                if tchunk is None:
                    tchunk = nt
                nchunk = nt // tchunk
                wname, src_m, dr_m, dt_m = main_spec
                wgname, src_g, dr_g, dt_g = gate_spec
                for j in range(nj):
                    main = pspool.tile([P, S], fp32, name=f"{wname}_m{j}",
                                       tag="pj_main", bufs=2)
                    gate = pspool.tile([P, S], fp32, name=f"{wname}_g{j}",
                                       tag="pj_gate", bufs=2)

                    def path(acc, wnm, src, dr, wdt, tag):
                        for ci in range(nchunk):
                            wt = wpool.tile([P, tchunk, P], wdt, tag=tag,
                                            name=f"w_{wnm}_{j}_{ci}",
                                            bufs=wbufs)
                            nc.sync.dma_start(
                                wt[:],
                                wd[wnm].ap()[j, :,
                                             ci * tchunk:(ci + 1) * tchunk,
                                             :])
                            if dr:
                                for pi in range(tchunk // 2):
                                    t = ci * tchunk + 2 * pi
                                    nc.tensor.matmul(
                                        acc[:], wt[:, 2 * pi:2 * pi + 2, :],
                                        src[:, t:t + 2, :],
                                        start=(t == 0), stop=(t == nt - 2),
                                        perf_mode=DR)
                            else:
                                for ti in range(tchunk):
                                    t = ci * tchunk + ti
                                    nc.tensor.matmul(
                                        acc[:], wt[:, ti, :], src[:, t, :],
                                        start=(t == 0), stop=(t == nt - 1))

                    path(main, wname, src_m, dr_m, dt_m, "wmain")
                    path(gate, wgname, src_g, dr_g, dt_g, "wgate")
                    sig = wpool.tile([P, S], bf16, tag="sig",
                                     name=f"sig_{wname}_{j}", bufs=3)
                    nc.scalar.activation(sig[:], gate[:], AF.Sigmoid,
                                         bias=bias[bgname][:, j:j + 1],
                                         scale=(INV if dr_g else 1.0))
                    epilogue(j, main, sig)

            # x2 outlives phases A-C (used by LN2 + MLP residual)
            with tc.tile_pool(name="x2p", bufs=1) as x2p:
              with tc.tile_pool(name="xt", bufs=1) as xtp:
                xt = xtp.tile([P, NT, S], fp32)
                xT_v = xT_d.ap().rearrange("(t p) s -> t p s", p=P)
                for t in range(NT):
                    nc.sync.dma_start(xt[:, t, :], xT_v[t])

                vN_bounce = dramp.tile([S, D], f8)
                k_bounce = dramp.tile([D, S], bf16)
                vgN = dramp.tile([GROUP * S, D], f8)
                kg = dramp.tile([GROUP * D, S], bf16)

                with tc.tile_pool(name="yp", bufs=1) as ypool:
                  with tc.tile_pool(name="qp", bufs=1) as qpool:
                    q = qpool.tile([P, NT, S], bf16)
                    vresp_cm = tc.tile_pool(name="vres", bufs=1)
                    vresp = vresp_cm.__enter__()
                    # V resident [k-part, kb, d] fp8; loads issued right
                    # after the V AllGather (ahead of the K collective on
                    # the GpSimd queue) so they overlap K/Q projections.
                    Vt = vresp.tile([P, NKB, D], f8)

                    with tc.tile_pool(name="hq", bufs=1) as hqp:
                        with (
                            tc.tile_pool(name="ln1tmp", bufs=1) as ln1tmp,
                            tc.tile_pool(name="ln1ps", bufs=1,
                                         space="PSUM") as ln1ps,
                        ):
                            (h1,) = ln_T(xt, "g1", "bt1", [(hqp, f8)],
                                         ln1tmp, ln1ps, "ln1")

                        # ---- V projection, N-layout, fp8 DoubleRow ----
                        with (
                            tc.tile_pool(name="wv", bufs=1) as wvp,
                            tc.tile_pool(name="vps", bufs=1,
                                         space="PSUM") as vps,
                        ):
                            TC = NT // 2
                            for n in range(4):
                                vmain = [vps.tile([P, S], fp32,
                                                  tag="v_main", bufs=4,
                                                  name=f"vm_{n}_{m}")
                                         for m in range(4)]
                                vgate = [vps.tile([P, S], fp32,
                                                  tag="v_gate", bufs=4,
                                                  name=f"vg_{n}_{m}")
                                         for m in range(4)]
                                for ci in range(2):
                                    wvt = wvp.tile([P, TC, 4 * P], f8,
                                                   tag="wv", bufs=2,
                                                   name=f"wv_{n}_{ci}")
                                    wgvt = wvp.tile([P, TC, 4 * P], f8,
                                                    tag="wgv", bufs=2,
                                                    name=f"wgv_{n}_{ci}")
                                    for ti in range(TC):
                                        t = ci * TC + ti
                                        nc.sync.dma_start(
                                            wvt[:, ti, :],
                                            wd["WvT"].ap()[t * P:(t + 1) * P,
                                                           n * S:(n + 1) * S])
                                        nc.sync.dma_start(
                                            wgvt[:, ti, :],
                                            wd["WgvT"].ap()[
                                                t * P:(t + 1) * P,
                                                n * S:(n + 1) * S])
                                    for m in range(4):
                                        for pi in range(TC // 2):
                                            t = ci * TC + 2 * pi
                                            nc.tensor.matmul(
                                                vmain[m][:],
                                                h1[:, t:t + 2,
                                                   m * P:(m + 1) * P],
                                                wvt[:, 2 * pi:2 * pi + 2, :],
                                                start=(t == 0), stop=False,
                                                perf_mode=DR)
                                        for pi in range(TC // 2):
                                            t = ci * TC + 2 * pi
                                            nc.tensor.matmul(
                                                vgate[m][:],
                                                h1[:, t:t + 2,
                                                   m * P:(m + 1) * P],
                                                wgvt[:, 2 * pi:2 * pi + 2, :],
                                                start=(t == 0), stop=False,
                                                perf_mode=DR)
                                for m in range(4):
                                    nc.tensor.matmul(
                                        vmain[m][:], ones_row[:],
                                        bvrow[:, n * S:(n + 1) * S],
                                        start=False, stop=True)
                                    nc.tensor.matmul(
                                        vgate[m][:], ones_row[:],
                                        bgvrow[:, n * S:(n + 1) * S],
                                        start=False, stop=True)
                                    vsig = wvp.tile([P, S], bf16,
                                                    tag="vsig", bufs=3,
                                                    name=f"vsig_{n}_{m}")
                                    nc.scalar.activation(vsig[:],
                                                         vgate[m][:],
                                                         AF.Sigmoid,
                                                         scale=INV)
                                    vout = wvp.tile([P, S], f8,
                                                    tag="vout", bufs=3,
                                                    name=f"vout_{n}_{m}")
                                    nc.vector.scalar_tensor_tensor(
                                        vout[:], vmain[m][:], INV, vsig[:],
                                        op0=ALU.mult, op1=ALU.mult)
                                    nc.scalar.dma_start(
                                        vN_bounce[m * P:(m + 1) * P,
                                                  n * S:(n + 1) * S],
                                        vout[:])

                        nc.gpsimd.collective_compute(
                            "AllGather", ALU.bypass, ins=[vN_bounce[:]],
                            outs=[vgN[:]], replica_groups=RG)
                        for kb in range(NKB):
                            nc.gpsimd.dma_start(
                                Vt[:, kb, :],
                                vgN[kb * P:(kb + 1) * P, :])

                        # ---- K projection (T-layout) + AllGather ----
                        with (
                            tc.tile_pool(name="wproj", bufs=1) as wpool,
                            tc.tile_pool(name="pjps", bufs=1,
                                         space="PSUM") as pjps,
                        ):
                            def k_epi(j, main, sig):
                                kv = wpool.tile([P, S], bf16, tag="kv_out",
                                                name=f"kv_k_{j}", bufs=3)
                                nc.vector.scalar_tensor_tensor(
                                    kv[:], main[:], bias["bk"][:, j:j + 1],
                                    sig[:], op0=ALU.add, op1=ALU.mult)
                                nc.scalar.dma_start(
                                    k_bounce[j * P:(j + 1) * P, :], kv[:])

                            proj_gated(NT, NT, ("Wk", h1, True, f8),
                                       ("Wgk", h1, True, f8), "bgk",
                                       wpool, pjps, k_epi)

                            nc.gpsimd.collective_compute(
                                "AllGather", ALU.bypass, ins=[k_bounce[:]],
                                outs=[kg[:]], replica_groups=RG)

                            def q_epi(j, main, sig):
                                nc.vector.scalar_tensor_tensor(
                                    q[:, j, :], main[:],
                                    bias["bq"][:, j:j + 1],
                                    sig[:], op0=ALU.add, op1=ALU.mult)

                            proj_gated(NT, NT, ("Wq", h1, True, f8),
                                       ("Wgq", h1, True, f8), "bgq",
                                       wpool, pjps, q_epi)

                    # ---- phase B: attention ----
                    with (
                        tc.tile_pool(name="kstream", bufs=2) as kpool,
                        tc.tile_pool(name="apool", bufs=4) as apool,
                        tc.tile_pool(name="atps", bufs=1,
                                     space="PSUM") as atps,
                    ):
                        y = ypool.tile([P, NT, S], f8)

                        head_state = {}

                        def finalize_head(h, Zp_h, Yp_h):
                            urow = rows.tile([1, S], fp32, name=f"u_{h}",
                                             tag="urow", bufs=2)
                            nc.vector.reciprocal(urow[:], Zp_h[:])
                            ubf = rows.tile([1, S], bf16, name=f"ubf_{h}",
                                            tag="ubf", bufs=2)
                            nc.vector.tensor_copy(ubf[:], urow[:])
                            Up = atps.tile([P, S], fp32, name=f"Up_{h}",
                                           tag="logits", bufs=4)
                            nc.tensor.matmul(Up[:], ones_row[:], ubf[:])
                            Us = apool.tile([P, S], bf16, tag="Us",
                                            name=f"Us_{h}")
                            nc.vector.tensor_copy(Us[:], Up[:])
                            nc.vector.tensor_tensor(y[:, h, :], Yp_h[:],
                                                    Us[:], op=ALU.mult)

                        for hh in range(H):
                            Kh = kpool.tile([P, NKB * P], bf16, tag="Kh",
                                            name=f"Kh_{hh}")
                            for s_ in range(GROUP):
                                nc.gpsimd.dma_start(
                                    Kh[:, s_ * S:(s_ + 1) * S],
                                    kg[s_ * D + hh * P:
                                       s_ * D + (hh + 1) * P, :])
                            Zp = atps.tile([1, S], fp32, name=f"Z_{hh}",
                                           tag="Zp", bufs=2)
                            Yp = atps.tile([P, S], fp32, name=f"Y_{hh}",
                                           tag="Yp", bufs=2)
                            ats = {}

                            def do_pair(pi, hh=hh, Kh=Kh, ats=ats):
                                At2 = apool.tile([P, 2, S], f8, tag="At2",
                                                 name=f"At2_{hh}_{pi}",
                                                 bufs=4)
                                for i in range(2):
                                    kb = 2 * pi + i
                                    Lp = atps.tile([P, S], fp32,
                                                   name=f"L_{hh}_{kb}",
                                                   tag="logits", bufs=4)
                                    nc.tensor.matmul(
                                        Lp[:], Kh[:, kb * P:(kb + 1) * P],
                                        q[:, hh, :])
                                    nc.scalar.activation(At2[:, i, :], Lp[:],
                                                         AF.Exp,
                                                         scale=EXPSCALE)
                                ats[pi] = At2

                            do_pair(0)
                            do_pair(1)
                            for pi in range(NPAIR):
                                if pi + 2 < NPAIR:
                                    do_pair(pi + 2)
                                At2 = ats.pop(pi)
                                nc.tensor.matmul(Zp[:], ones2[:, :, 0:1],
                                                 At2[:],
                                                 start=(pi == 0),
                                                 stop=(pi == NPAIR - 1),
                                                 perf_mode=DR)
                                nc.tensor.matmul(
                                    Yp[:],
                                    Vt[:, 2 * pi:2 * pi + 2,
                                       hh * P:(hh + 1) * P],
                                    At2[:],
                                    start=(pi == 0),
                                    stop=(pi == NPAIR - 1),
                                    perf_mode=DR)
                                if pi == 2 and hh > 0:
                                    finalize_head(hh - 1,
                                                  *head_state[hh - 1])
                            head_state[hh] = (Zp, Yp)
                        finalize_head(H - 1, *head_state[H - 1])
                    vresp_cm.__exit__(None, None, None)

                  # ---- phase C: o-proj + residual (LN2 stats inline) ----
                  x2 = x2p.tile([P, NT, S], fp32, name="x2")
                  ln2ps_cm = tc.tile_pool(name="ln2ps", bufs=1,
                                          space="PSUM")
                  ln2ps = ln2ps_cm.__enter__()
                  S1_2 = ln2ps.tile([1, S], fp32, name="ln2_S1",
                                    tag="ln_S1")
                  S2_2 = ln2ps.tile([1, S], fp32, name="ln2_S2",
                                    tag="ln_S2")
                  with (
                      tc.tile_pool(name="wproj2", bufs=1) as wpool2,
                      tc.tile_pool(name="pj2ps", bufs=1,
                                   space="PSUM") as pj2ps,
                  ):
                      def o_epi(j, main, sig):
                          tmp = wpool2.tile([P, S], fp32, tag="o_tmp",
                                            name=f"o_tmp_{j}", bufs=3)
                          nc.vector.scalar_tensor_tensor(
                              tmp[:], main[:], bias["bo"][:, j:j + 1],
                              sig[:], op0=ALU.add, op1=ALU.mult)
                          nc.vector.scalar_tensor_tensor(
                              x2[:, j, :], tmp[:], INV, xt[:, j, :],
                              op0=ALU.mult, op1=ALU.add)
                          if j % 2 == 1:
                              ln_stats_pair(S1_2, S2_2, x2[:, j - 1, :],
                                            x2[:, j, :], wpool2, j // 2,
                                            NT // 2, "ln2")

                      proj_gated(NT, NT, ("Wo", y, True, f8),
                                 ("Wgo", y, True, f8), "bgo",
                                 wpool2, pj2ps, o_epi)

              # ---- phase D: LN2 + MLP ----
              with tc.tile_pool(name="midp", bufs=1) as midp:
                  mid_bf = midp.tile([P, NF, S], bf16)
                  mid_f8 = midp.tile([P, NF, S], f8)
                  with tc.tile_pool(name="h2p", bufs=1) as h2p:
                      with tc.tile_pool(name="ln2tmp", bufs=1) as ln2tmp:
                          (h2_f8,) = ln_T(x2, "g2", "bt2", [(h2p, f8)],
                                          ln2tmp, ln2ps, "ln2",
                                          stats=(S1_2, S2_2))
                      ln2ps_cm.__exit__(None, None, None)

                      with (
                          tc.tile_pool(name="wmlp1", bufs=1) as wm1,
                          tc.tile_pool(name="m1ps", bufs=1,
                                       space="PSUM") as m1ps,
                      ):
                          def mid_epi(j, main, sig):
                              tmp = wm1.tile([P, S], fp32, tag="mid_tmp",
                                             name=f"mid_tmp_{j}", bufs=3)
                              nc.vector.scalar_tensor_tensor(
                                  tmp[:], main[:],
                                  bias["bin"][:, j:j + 1], sig[:],
                                  op0=ALU.add, op1=ALU.mult)
                              nc.scalar.activation(mid_bf[:, j, :], tmp[:],
                                                   AF.Gelu, scale=INV)
                              nc.scalar.activation(mid_f8[:, j, :], tmp[:],
                                                   AF.Gelu, scale=INV)

                          proj_gated(NT, NF, ("Win", h2_f8, True, f8),
                                     ("Wgin", h2_f8, True, f8), "bgin",
                                     wm1, m1ps, mid_epi)

                  with (
                      tc.tile_pool(name="wmlp2", bufs=1) as wm2,
                      tc.tile_pool(name="m2ps", bufs=1,
                                   space="PSUM") as m2ps,
                  ):
                      def out_epi(j, main, sig):
                          tmp = wm2.tile([P, S], fp32, tag="out_tmp",
                                         name=f"out_tmp_{j}", bufs=3)
                          nc.vector.scalar_tensor_tensor(
                              tmp[:], main[:], bias["bout"][:, j:j + 1],
                              sig[:], op0=ALU.add, op1=ALU.mult)
                          outf = wm2.tile([P, S], fp32, tag="out_f",
                                          name=f"out_f_{j}", bufs=3)
                          nc.vector.tensor_tensor(outf[:], tmp[:],
                                                  x2[:, j, :], op=ALU.add)
                          nc.sync.dma_start(
                              out_d.ap()[j * P:(j + 1) * P, :], outf[:])

                      proj_gated(NF, NT, ("Wout", mid_bf, False, bf16),
                                 ("Wgout", mid_f8, True, f8), "bgout",
                                 wm2, m2ps, out_epi, tchunk=32, wbufs=2)

    nc.compile()
    return nc


def _prep_shared_inputs(inputs):
    m = {}
    # fp8 weights stored at 64x (T-layout tiles)
    for nm, w in (("Wq", "W_q"), ("Wgq", "Wg_q"), ("Wk", "W_k"),
                  ("Wgk", "Wg_k"), ("Wo", "W_o"), ("Wgo", "Wg_o"),
                  ("Win", "W_in"), ("Wgin", "Wg_in"), ("Wgout", "Wg_out")):
        m[nm] = _w_tiled(np.asarray(inputs[w]), WS, _F8)
    # bf16 main-path mlp-out weights at 1x
    m["Wout"] = _w_tiled(np.asarray(inputs["W_out"]), 1.0, _BF)
    m["WvT"] = np.ascontiguousarray(
        (np.asarray(inputs["W_v"]).T * WS).astype(_F8))
    m["WgvT"] = np.ascontiguousarray(
        (np.asarray(inputs["Wg_v"]).T * WS).astype(_F8))
    m["bvrow"] = (np.asarray(inputs["b_v"]) * WS).astype(_BF).reshape(1, D)
    m["bgvrow"] = (np.asarray(inputs["bg_v"]) * WS).astype(_BF).reshape(1, D)
    for nm, bn, sc in (("bq", "b_q", WS), ("bgq", "bg_q", 1.0),
                       ("bk", "b_k", WS), ("bgk", "bg_k", 1.0),
                       ("bo", "b_o", WS), ("bgo", "bg_o", 1.0),
                       ("bin", "b_in", WS), ("bgin", "bg_in", 1.0),
                       ("bout", "b_out", 1.0), ("bgout", "bg_out", 1.0),
                       ("g1", "ln1_g", 1.0), ("bt1", "ln1_b", 1.0),
                       ("g2", "ln2_g", 1.0), ("bt2", "ln2_b", 1.0)):
        m[nm] = _b_cols(np.asarray(inputs[bn]), sc)
    return m


def _install_trace_shim():
    """Provide antenv.axon_hooks (NTFF profiling) if the image lacks it."""
    import contextlib
    import ctypes
    import types

    try:
        import antenv.axon_hooks  # noqa: F401
        return
    except ImportError:
        pass
    try:
        import antenv
    except ImportError:
        return
    so_path = "/opt/axon/libaxon_pjrt.so"
    try:
        lib = ctypes.CDLL(so_path)
    except OSError:
        return
    if not hasattr(lib, "axon_start_nrt_profile"):
        return
    lib.axon_start_nrt_profile.argtypes = [ctypes.POINTER(ctypes.c_int64),
                                           ctypes.c_size_t]
    lib.axon_start_nrt_profile.restype = ctypes.c_int64
    lib.axon_stop_nrt_profile.argtypes = [ctypes.c_char_p]
    lib.axon_stop_nrt_profile.restype = ctypes.c_int64

    @contextlib.contextmanager
    def hook(output_dir, device_ids):
        import jax

        jax.devices()
        if device_ids:
            ids = (ctypes.c_int64 * len(device_ids))(*device_ids)
            rc = lib.axon_start_nrt_profile(ids, len(device_ids))
        else:
            rc = lib.axon_start_nrt_profile(None, 0)
        if rc != 0:
            raise RuntimeError(f"axon_start_nrt_profile rc={rc}")
        try:
            yield
        finally:
            n = lib.axon_stop_nrt_profile(str(output_dir).encode())
            print(f"profile: {n} ntff file(s) in {output_dir}",
                  file=sys.stderr)

    mod = types.ModuleType("antenv.axon_hooks")
    mod.get_axon_ntff_profile_hook = lambda: hook
    mod.set_axon_ntff_profile_hook = lambda h: None
    sys.modules["antenv.axon_hooks"] = mod
    antenv.axon_hooks = mod


LAST_RESULTS = None


def kernel(_trace=False, **inputs):
    global _COMPILED, LAST_RESULTS
    from concourse import bass_utils

    if _trace:
        _install_trace_shim()

    if _COMPILED is None:
        _COMPILED = _build()
    nc = _COMPILED

    shared = _prep_shared_inputs(inputs)
    x = np.asarray(inputs["x"], dtype=np.float32)  # [B, T, D]
    in_maps = []
    for c in range(N_CORES):
        g, s = divmod(c, GROUP)
        xT_c = np.ascontiguousarray(x[g, s * S:(s + 1) * S, :].T)
        m = dict(shared)
        m["xT"] = xT_c
        in_maps.append(m)

    LAST_RESULTS = bass_utils.run_bass_kernel_spmd(
        nc, in_maps, core_ids=list(range(N_CORES)), trace=_trace)

    out = np.empty((B, T, D), dtype=np.float32)
    for c in range(N_CORES):
        g, s = divmod(c, GROUP)
        out[g, s * S:(s + 1) * S, :] = LAST_RESULTS.results[c]["outT"].T
    return out


# revision 26
# speedup vs baseline: 1.5856x; 1.0221x over previous
"""Trainium2 Bass kernel for nn_DGEBlock (dense transformer block with
MoE-gated linears), distributed over 8 NeuronCores.

Sharding: data-parallel over batch (2 groups of 4 cores) x sequence-parallel
over tokens within each batch (512 tokens per core). Weights are replicated.
Activations live feature-major ("T-layout": [d, tok]) in SBUF so projections
are lhsT=W^T-tile @ rhs=activation with no activation transposes. V is
projected in token-major (N-)layout directly so attention's PV matmuls need
no transposes either.

Precision scheme (fp8 DoubleRow halves the matmul count where used):
  - q/k/v/o projections: main+gate both fp8e4 DoubleRow (weights stored
    64x in e4m3; epilogues fold the 1/64 into activation scales).
  - MLP in/out: MAIN path stays bf16 (accuracy: errors there land on the
    residual stream through the widest matrices); GATE path fp8 DoubleRow
    (sigmoid compresses quantization noise).
  - Attention: At (exp logits) and V stored fp8; PV and the softmax
    denominator (Z) matmuls run DoubleRow over key-block pairs; QK stays
    bf16 (contraction is only 128 so DoubleRow can't help).
  - LayerNorm stats summed from fp8 copies via DoubleRow ones-matmuls
    (2048-way averaging makes this noise negligible).
The only collectives are two 4-rank AllGathers (V in fp8, K in bf16).
Output is returned token-sharded and reassembled on host.
"""

import sys

for _p in ("/opt/trn_rl_repo",):
    if _p not in sys.path:
        sys.path.append(_p)

import numpy as np
import ml_dtypes

# ---------------------------------------------------------------- constants
B = 2
T = 2048
D = 2048
H = 16
HD = 128
FF = 4 * D  # 8192
EPS = 1e-5

N_CORES = 8
GROUP = 4  # cores per batch group (sequence-parallel degree)
S = T // GROUP  # tokens per core = 512
P = 128
NT = D // P  # 16 feature tiles
NF = FF // P  # 64 hidden tiles
NKB = T // P  # 16 key blocks per batch
NPAIR = NKB // 2
ISCALE = 1.0 / float(np.sqrt(HD))

WS = 64.0  # fp8 weight pre-scale (keeps 0.02-std weights out of subnormals)
INV = 1.0 / WS
EXPSCALE = ISCALE / 16.0  # q and k are both stored fp8 at 4x

RG = [[0, 1, 2, 3], [4, 5, 6, 7]]

_BF = ml_dtypes.bfloat16
_F8 = ml_dtypes.float8_e4m3

_COMPILED = None


# ------------------------------------------------------------- host prep
def _w_tiled(W, scale, dt):
    """W [dout, din] -> [nj, 128, nt, 128] such that
    out[j, p, t, jc] == scale*W[j*128+jc, t*128+p]  (= W^T tile (t, j))."""
    dout, din = W.shape
    nj, nt = dout // P, din // P
    return np.ascontiguousarray(
        (W.reshape(nj, P, nt, P) * scale).transpose(0, 3, 2, 1).astype(dt)
    )


def _b_cols(b, scale=1.0):
    """b [dout] -> [128, nj] fp32: column j holds scale*b[j*128:(j+1)*128]."""
    nj = b.shape[0] // P
    return np.ascontiguousarray((b * scale).reshape(nj, P).T.astype(np.float32))


# ------------------------------------------------------------- device build
def _build():
    from concourse import bacc, tile, mybir

    fp32 = mybir.dt.float32
    bf16 = mybir.dt.bfloat16
    f8 = mybir.dt.float8e4
    AF = mybir.ActivationFunctionType
    ALU = mybir.AluOpType
    DR = mybir.MatmulPerfMode.DoubleRow

    nc = bacc.Bacc("TRN2", target_bir_lowering=False, debug=False,
                   num_devices=N_CORES)

    # ---- I/O tensors
    xT_d = nc.dram_tensor("xT", [D, S], fp32, kind="ExternalInput")
    wd = {}
    for nm in ("Wq", "Wgq", "Wk", "Wgk", "Wo", "Wgo"):
        wd[nm] = nc.dram_tensor(nm, [NT, P, NT, P], f8, kind="ExternalInput")
    wd["Win"] = nc.dram_tensor("Win", [NF, P, NT, P], f8,
                               kind="ExternalInput")
    wd["Wgin"] = nc.dram_tensor("Wgin", [NF, P, NT, P], f8,
                                kind="ExternalInput")
    wd["Wout"] = nc.dram_tensor("Wout", [NT, P, NF, P], bf16,
                                kind="ExternalInput")
    wd["Wgout"] = nc.dram_tensor("Wgout", [NT, P, NF, P], f8,
                                 kind="ExternalInput")
    # V projection runs in N-layout: plain W^T [din, dout] + bias rows
    wd["WvT"] = nc.dram_tensor("WvT", [D, D], f8, kind="ExternalInput")
    wd["WgvT"] = nc.dram_tensor("WgvT", [D, D], f8, kind="ExternalInput")
    bvrow_d = nc.dram_tensor("bvrow", [1, D], bf16, kind="ExternalInput")
    bgvrow_d = nc.dram_tensor("bgvrow", [1, D], bf16, kind="ExternalInput")
    bd = {}
    for nm in ("bq", "bgq", "bk", "bgk", "bo", "bgo",
               "bout", "bgout", "g1", "bt1", "g2", "bt2"):
        bd[nm] = nc.dram_tensor(nm, [P, NT], fp32, kind="ExternalInput")
    for nm in ("bin", "bgin"):
        bd[nm] = nc.dram_tensor(nm, [P, NF], fp32, kind="ExternalInput")
    out_d = nc.dram_tensor("outT", [D, S], fp32, kind="ExternalOutput")

    with tile.TileContext(nc) as tc:
        with (
            tc.tile_pool(name="const", bufs=1) as constp,
            tc.tile_pool(name="bias", bufs=1) as biasp,
            tc.tile_pool(name="rows", bufs=1) as rows,
            tc.tile_pool(name="dram", bufs=1, space="DRAM") as dramp,
        ):
            ones_col = constp.tile([P, 1], bf16)
            nc.vector.memset(ones_col[:], 1.0)
            ones_row = constp.tile([1, P], bf16)
            nc.vector.memset(ones_row[:], 1.0)
            # fp8 "ones" pair for DoubleRow contractions with unit weights;
            # [P, 2, 16] so the pair-step is 16B (DoubleRow AP constraint)
            ones2 = constp.tile([P, 2, 16], f8)
            nc.vector.memset(ones2[:], 1.0)
            eps_t = constp.tile([1, 1], fp32)
            nc.vector.memset(eps_t[:], EPS)
            bvrow = constp.tile([1, D], bf16)
            nc.sync.dma_start(bvrow[:], bvrow_d.ap())
            bgvrow = constp.tile([1, D], bf16)
            nc.sync.dma_start(bgvrow[:], bgvrow_d.ap())

            bias = {}
            for nm in bd:
                ncols = NF if nm in ("bin", "bgin") else NT
                btile = biasp.tile([P, ncols], fp32, name=f"bias_{nm}")
                nc.sync.dma_start(btile[:], bd[nm].ap())
                bias[nm] = btile

            # ---------- helpers ----------
            def ln_stats_pair(S1, S2, src0, src1, tmpool, pi, npair, name):
                """Accumulate sum/sum-sq of one feature-tile pair into the
                S1/S2 psums via fp8 DoubleRow ones-matmuls."""
                xp = tmpool.tile([P, 2, S], f8, tag="ln_x8", bufs=2,
                                 name=f"{name}_x8_{pi}")
                sp = tmpool.tile([P, 2, S], f8, tag="ln_sq", bufs=2,
                                 name=f"{name}_sq_{pi}")
                for i, srct in enumerate((src0, src1)):
                    nc.vector.tensor_copy(xp[:, i, :], srct)
                    nc.scalar.activation(sp[:, i, :], srct, AF.Square)
                nc.tensor.matmul(S1[:], ones2[:, :, 0:1], xp[:],
                                 start=(pi == 0), stop=(pi == npair - 1),
                                 perf_mode=DR)
                nc.tensor.matmul(S2[:], ones2[:, :, 0:1], sp[:],
                                 start=(pi == 0), stop=(pi == npair - 1),
                                 perf_mode=DR)

            def ln_T(src, gname, bname, outs, tmpool, psln, name,
                     stats=None):
                """LayerNorm over the feature dim of a T-layout activation.

                src: SBUF tile [128, NT, S] fp32.  outs: list of
                (pool, dtype) -> returns one [128, NT, S] tile per entry.
                Stats via fp8 DoubleRow ones-matmuls (contract over
                partitions) unless passed precomputed; per-token scale/shift
                rows are broadcast to [128, S] via rank-1 matmuls.  The
                apply chain alternates Vector/GpSimd per tile to halve the
                serial latency.
                """
                if stats is None:
                    S1 = psln.tile([1, S], fp32, name=f"{name}_S1",
                                   tag="ln_S1")
                    S2 = psln.tile([1, S], fp32, name=f"{name}_S2",
                                   tag="ln_S2")
                    for pi in range(NT // 2):
                        t = 2 * pi
                        ln_stats_pair(S1, S2, src[:, t, :], src[:, t + 1, :],
                                      tmpool, pi, NT // 2, name)
                else:
                    S1, S2 = stats

                def row(nm, dt=fp32):
                    return rows.tile([1, S], dt, name=f"{name}_{nm}",
                                     tag=f"ln_{nm}")

                mean = row("mean")
                nc.vector.tensor_scalar_mul(mean[:], S1[:], 1.0 / D)
                m2 = row("m2")
                nc.vector.tensor_scalar_mul(m2[:], S2[:], 1.0 / D)
                msq = row("msq")
                nc.vector.tensor_tensor(msq[:], mean[:], mean[:],
                                        op=ALU.mult)
                var = row("var")
                nc.vector.tensor_tensor(var[:], m2[:], msq[:],
                                        op=ALU.subtract)
                std = row("std")
                nc.scalar.activation(std[:], var[:], AF.Sqrt,
                                     bias=eps_t[:])
                rstd = row("rstd")
                nc.vector.reciprocal(rstd[:], std[:])
                rstd_bf = row("rstdbf", bf16)
                nc.vector.tensor_copy(rstd_bf[:], rstd[:])
                mr_bf = row("mrbf", bf16)
                nc.vector.tensor_tensor(mr_bf[:], mean[:], rstd[:],
                                        op=ALU.mult)
                Ab_p = psln.tile([P, S], fp32, name=f"{name}_Abp",
                                 tag="ln_Abp")
                nc.tensor.matmul(Ab_p[:], ones_row[:], rstd_bf[:])
                Bb_p = psln.tile([P, S], fp32, name=f"{name}_Bbp",
                                 tag="ln_Bbp")
                nc.tensor.matmul(Bb_p[:], ones_row[:], mr_bf[:])
                Ab = tmpool.tile([P, S], fp32, name=f"{name}_Ab")
                nc.vector.tensor_copy(Ab[:], Ab_p[:])
                Bb = tmpool.tile([P, S], fp32, name=f"{name}_Bb")
                nc.vector.tensor_copy(Bb[:], Bb_p[:])
                hs = [pool.tile([P, NT, S], dt, name=f"{name}_h{i}")
                      for i, (pool, dt) in enumerate(outs)]
                for t in range(NT):
                    eng = nc.vector if t % 2 == 0 else nc.gpsimd
                    tmp = tmpool.tile([P, S], fp32, name=f"{name}_t0_{t}",
                                      tag="ln_t0", bufs=4)
                    eng.tensor_tensor(tmp[:], src[:, t, :], Ab[:],
                                      op=ALU.mult)
                    tmp2 = tmpool.tile([P, S], fp32, name=f"{name}_t1_{t}",
                                       tag="ln_t1", bufs=4)
                    eng.tensor_tensor(tmp2[:], tmp[:], Bb[:],
                                      op=ALU.subtract)
                    for h in hs:
                        nc.scalar.activation(h[:, t, :], tmp2[:], AF.Identity,
                                             bias=bias[bname][:, t:t + 1],
                                             scale=bias[gname][:, t:t + 1])
                return hs

            def proj_gated(nt, nj, main_spec, gate_spec, bgname, wpool,
                           pspool, epilogue, tchunk=None, wbufs=3):
                """Gated projection in T-layout.  spec = (wname, src, dr,
                wdtype).  dr=True runs fp8 DoubleRow over k-tile pairs."""
                if tchunk is None:
                    tchunk = nt
                nchunk = nt // tchunk
                wname, src_m, dr_m, dt_m = main_spec
                wgname, src_g, dr_g, dt_g = gate_spec
                for j in range(nj):
                    main = pspool.tile([P, S], fp32, name=f"{wname}_m{j}",
                                       tag="pj_main", bufs=2)
                    gate = pspool.tile([P, S], fp32, name=f"{wname}_g{j}",
                                       tag="pj_gate", bufs=2)

                    def path(acc, wnm, src, dr, wdt, tag):
                        for ci in range(nchunk):
                            wt = wpool.tile([P, tchunk, P], wdt, tag=tag,
                                            name=f"w_{wnm}_{j}_{ci}",
                                            bufs=wbufs)
                            nc.sync.dma_start(
                                wt[:],
                                wd[wnm].ap()[j, :,
                                             ci * tchunk:(ci + 1) * tchunk,
                                             :])
                            if dr:
                                for pi in range(tchunk // 2):
                                    t = ci * tchunk + 2 * pi
                                    nc.tensor.matmul(
                                        acc[:], wt[:, 2 * pi:2 * pi + 2, :],
                                        src[:, t:t + 2, :],
                                        start=(t == 0), stop=(t == nt - 2),
                                        perf_mode=DR)
                            else:
                                for ti in range(tchunk):
                                    t = ci * tchunk + ti
                                    nc.tensor.matmul(
                                        acc[:], wt[:, ti, :], src[:, t, :],
                                        start=(t == 0), stop=(t == nt - 1))

                    path(main, wname, src_m, dr_m, dt_m, "wmain")
                    path(gate, wgname, src_g, dr_g, dt_g, "wgate")
                    sig = wpool.tile([P, S], bf16, tag="sig",
                                     name=f"sig_{wname}_{j}", bufs=3)
                    nc.scalar.activation(sig[:], gate[:], AF.Sigmoid,
                                         bias=bias[bgname][:, j:j + 1],
                                         scale=(INV if dr_g else 1.0))
                    epilogue(j, main, sig)

            # x2 outlives phases A-C (used by LN2 + MLP residual)
            with tc.tile_pool(name="x2p", bufs=1) as x2p:
              with tc.tile_pool(name="xt", bufs=1) as xtp:
                xt = xtp.tile([P, NT, S], fp32)
                xT_v = xT_d.ap().rearrange("(t p) s -> t p s", p=P)
                for t in range(NT):
                    nc.sync.dma_start(xt[:, t, :], xT_v[t])

                vN_bounce = dramp.tile([S, D], f8)
                k_bounce = dramp.tile([D, S], f8)
                vgN = dramp.tile([GROUP * S, D], f8)
                kg = dramp.tile([GROUP * D, S], f8)

                with tc.tile_pool(name="yp", bufs=1) as ypool:
                  with tc.tile_pool(name="qp", bufs=1) as qpool:
                    q = qpool.tile([P, NT, S], f8)
                    vresp_cm = tc.tile_pool(name="vres", bufs=1)
                    vresp = vresp_cm.__enter__()
                    # V resident [k-part, kb, d] fp8; loads issued right
                    # after the V AllGather (ahead of the K collective on
                    # the GpSimd queue) so they overlap the Q projection.
                    Vt = vresp.tile([P, NKB, D], f8)

                    with tc.tile_pool(name="hq", bufs=1) as hqp:
                        with (
                            tc.tile_pool(name="ln1tmp", bufs=1) as ln1tmp,
                            tc.tile_pool(name="ln1ps", bufs=1,
                                         space="PSUM") as ln1ps,
                        ):
                            (h1,) = ln_T(xt, "g1", "bt1", [(hqp, f8)],
                                         ln1tmp, ln1ps, "ln1")

                        # Phase order: K-proj -> CC(K) -> V-proj -> CC(V)
                        # -> Q-proj, so each AllGather overlaps the next
                        # projection's matmul stream.
                        with tc.tile_pool(name="wproj", bufs=1) as wpool:
                          with tc.tile_pool(name="pjps", bufs=1,
                                            space="PSUM") as pjps:
                            def k_epi(j, main, sig):
                                kv64 = wpool.tile([P, S], fp32, tag="kv64",
                                                  name=f"kv64_{j}", bufs=3)
                                nc.vector.scalar_tensor_tensor(
                                    kv64[:], main[:],
                                    bias["bk"][:, j:j + 1],
                                    sig[:], op0=ALU.add, op1=ALU.mult)
                                kv = wpool.tile([P, S], f8, tag="kv_out",
                                                name=f"kv_k_{j}", bufs=3)
                                nc.vector.tensor_scalar_mul(kv[:], kv64[:],
                                                            1.0 / 16.0)
                                nc.scalar.dma_start(
                                    k_bounce[j * P:(j + 1) * P, :], kv[:])

                            proj_gated(NT, NT, ("Wk", h1, True, f8),
                                       ("Wgk", h1, True, f8), "bgk",
                                       wpool, pjps, k_epi)

                            nc.gpsimd.collective_compute(
                                "AllGather", ALU.bypass, ins=[k_bounce[:]],
                                outs=[kg[:]], replica_groups=RG)

                          # ---- V projection, N-layout, fp8 DoubleRow ----
                          with (
                            tc.tile_pool(name="wv", bufs=1) as wvp,
                            tc.tile_pool(name="vps", bufs=1,
                                         space="PSUM") as vps,
                          ):
                            TC = NT // 2
                            for n in range(4):
                                vmain = [vps.tile([P, S], fp32,
                                                  tag="v_main", bufs=4,
                                                  name=f"vm_{n}_{m}")
                                         for m in range(4)]
                                vgate = [vps.tile([P, S], fp32,
                                                  tag="v_gate", bufs=4,
                                                  name=f"vg_{n}_{m}")
                                         for m in range(4)]
                                for ci in range(2):
                                    wvt = wvp.tile([P, TC, 4 * P], f8,
                                                   tag="wv", bufs=2,
                                                   name=f"wv_{n}_{ci}")
                                    wgvt = wvp.tile([P, TC, 4 * P], f8,
                                                    tag="wgv", bufs=2,
                                                    name=f"wgv_{n}_{ci}")
                                    for ti in range(TC):
                                        t = ci * TC + ti
                                        nc.sync.dma_start(
                                            wvt[:, ti, :],
                                            wd["WvT"].ap()[t * P:(t + 1) * P,
                                                           n * S:(n + 1) * S])
                                        nc.sync.dma_start(
                                            wgvt[:, ti, :],
                                            wd["WgvT"].ap()[
                                                t * P:(t + 1) * P,
                                                n * S:(n + 1) * S])
                                    for m in range(4):
                                        for pi in range(TC // 2):
                                            t = ci * TC + 2 * pi
                                            nc.tensor.matmul(
                                                vmain[m][:],
                                                h1[:, t:t + 2,
                                                   m * P:(m + 1) * P],
                                                wvt[:, 2 * pi:2 * pi + 2, :],
                                                start=(t == 0), stop=False,
                                                perf_mode=DR)
                                        for pi in range(TC // 2):
                                            t = ci * TC + 2 * pi
                                            nc.tensor.matmul(
                                                vgate[m][:],
                                                h1[:, t:t + 2,
                                                   m * P:(m + 1) * P],
                                                wgvt[:, 2 * pi:2 * pi + 2, :],
                                                start=(t == 0), stop=False,
                                                perf_mode=DR)
                                for m in range(4):
                                    nc.tensor.matmul(
                                        vmain[m][:], ones_row[:],
                                        bvrow[:, n * S:(n + 1) * S],
                                        start=False, stop=True)
                                    nc.tensor.matmul(
                                        vgate[m][:], ones_row[:],
                                        bgvrow[:, n * S:(n + 1) * S],
                                        start=False, stop=True)
                                    vsig = wvp.tile([P, S], bf16,
                                                    tag="vsig", bufs=3,
                                                    name=f"vsig_{n}_{m}")
                                    nc.scalar.activation(vsig[:],
                                                         vgate[m][:],
                                                         AF.Sigmoid,
                                                         scale=INV)
                                    vout = wvp.tile([P, S], f8,
                                                    tag="vout", bufs=3,
                                                    name=f"vout_{n}_{m}")
                                    nc.vector.scalar_tensor_tensor(
                                        vout[:], vmain[m][:], INV, vsig[:],
                                        op0=ALU.mult, op1=ALU.mult)
                                    nc.scalar.dma_start(
                                        vN_bounce[m * P:(m + 1) * P,
                                                  n * S:(n + 1) * S],
                                        vout[:])

                            nc.gpsimd.collective_compute(
                                "AllGather", ALU.bypass, ins=[vN_bounce[:]],
                                outs=[vgN[:]], replica_groups=RG)
                            for kb in range(NKB):
                                nc.gpsimd.dma_start(
                                    Vt[:, kb, :],
                                    vgN[kb * P:(kb + 1) * P, :])

                          # ---- Q projection ----
                          with tc.tile_pool(name="pjps2", bufs=1,
                                            space="PSUM") as pjps2:
                            def q_epi(j, main, sig):
                                q64 = wpool.tile([P, S], fp32, tag="kv64",
                                                 name=f"q64_{j}", bufs=3)
                                nc.vector.scalar_tensor_tensor(
                                    q64[:], main[:],
                                    bias["bq"][:, j:j + 1],
                                    sig[:], op0=ALU.add, op1=ALU.mult)
                                nc.vector.tensor_scalar_mul(
                                    q[:, j, :], q64[:], 1.0 / 16.0)

                            proj_gated(NT, NT, ("Wq", h1, True, f8),
                                       ("Wgq", h1, True, f8), "bgq",
                                       wpool, pjps2, q_epi)

                    # o-proj weight pool opened BEFORE the attention pools
                    # so its SBUF range is disjoint from them -- its first
                    # weight DMAs then prefetch during attention instead of
                    # waiting for attention buffers to free.
                    wpool2_cm = tc.tile_pool(name="wproj2", bufs=1)
                    wpool2 = wpool2_cm.__enter__()

                    # ---- phase B: attention ----
                    with (
                        tc.tile_pool(name="kstream", bufs=2) as kpool,
                        tc.tile_pool(name="apool", bufs=4) as apool,
                        tc.tile_pool(name="atps", bufs=1,
                                     space="PSUM") as atps,
                    ):
                        y = ypool.tile([P, NT, S], f8)

                        head_state = {}

                        def finalize_head(h, Zp_h, Yp_h):
                            urow = rows.tile([1, S], fp32, name=f"u_{h}",
                                             tag="urow", bufs=2)
                            nc.vector.reciprocal(urow[:], Zp_h[:])
                            ubf = rows.tile([1, S], bf16, name=f"ubf_{h}",
                                            tag="ubf", bufs=2)
                            nc.vector.tensor_copy(ubf[:], urow[:])
                            Up = atps.tile([P, S], fp32, name=f"Up_{h}",
                                           tag="logits", bufs=4)
                            nc.tensor.matmul(Up[:], ones_row[:], ubf[:])
                            Us = apool.tile([P, S], bf16, tag="Us",
                                            name=f"Us_{h}")
                            nc.vector.tensor_copy(Us[:], Up[:])
                            nc.vector.tensor_tensor(y[:, h, :], Yp_h[:],
                                                    Us[:], op=ALU.mult)

                        for hh in range(H):
                            Kh = kpool.tile([P, NKB * P], f8, tag="Kh",
                                            name=f"Kh_{hh}")
                            for s_ in range(GROUP):
                                nc.gpsimd.dma_start(
                                    Kh[:, s_ * S:(s_ + 1) * S],
                                    kg[s_ * D + hh * P:
                                       s_ * D + (hh + 1) * P, :])
                            Zp = atps.tile([1, S], fp32, name=f"Z_{hh}",
                                           tag="Zp", bufs=2)
                            Yp = atps.tile([P, S], fp32, name=f"Y_{hh}",
                                           tag="Yp", bufs=2)
                            ats = {}

                            def do_pair(pi, hh=hh, Kh=Kh, ats=ats):
                                At2 = apool.tile([P, 2, S], f8, tag="At2",
                                                 name=f"At2_{hh}_{pi}",
                                                 bufs=4)
                                for i in range(2):
                                    kb = 2 * pi + i
                                    Lp = atps.tile([P, S], fp32,
                                                   name=f"L_{hh}_{kb}",
                                                   tag="logits", bufs=4)
                                    nc.tensor.matmul(
                                        Lp[:], Kh[:, kb * P:(kb + 1) * P],
                                        q[:, hh, :])
                                    nc.scalar.activation(At2[:, i, :], Lp[:],
                                                         AF.Exp,
                                                         scale=EXPSCALE)
                                ats[pi] = At2

                            do_pair(0)
                            do_pair(1)
                            for pi in range(NPAIR):
                                if pi + 2 < NPAIR:
                                    do_pair(pi + 2)
                                At2 = ats.pop(pi)
                                nc.tensor.matmul(Zp[:], ones2[:, :, 0:1],
                                                 At2[:],
                                                 start=(pi == 0),
                                                 stop=(pi == NPAIR - 1),
                                                 perf_mode=DR)
                                nc.tensor.matmul(
                                    Yp[:],
                                    Vt[:, 2 * pi:2 * pi + 2,
                                       hh * P:(hh + 1) * P],
                                    At2[:],
                                    start=(pi == 0),
                                    stop=(pi == NPAIR - 1),
                                    perf_mode=DR)
                                if pi == 2 and hh > 0:
                                    finalize_head(hh - 1,
                                                  *head_state[hh - 1])
                            head_state[hh] = (Zp, Yp)
                        finalize_head(H - 1, *head_state[H - 1])

                    # ---- phase C: o-proj + residual (LN2 stats inline) ---
                    x2 = x2p.tile([P, NT, S], fp32, name="x2")
                    ln2ps_cm = tc.tile_pool(name="ln2ps", bufs=1,
                                            space="PSUM")
                    ln2ps = ln2ps_cm.__enter__()
                    S1_2 = ln2ps.tile([1, S], fp32, name="ln2_S1",
                                      tag="ln_S1")
                    S2_2 = ln2ps.tile([1, S], fp32, name="ln2_S2",
                                      tag="ln_S2")
                    with tc.tile_pool(name="pj2ps", bufs=1,
                                      space="PSUM") as pj2ps:
                        def o_epi(j, main, sig):
                            tmp = wpool2.tile([P, S], fp32, tag="o_tmp",
                                              name=f"o_tmp_{j}", bufs=3)
                            nc.vector.scalar_tensor_tensor(
                                tmp[:], main[:], bias["bo"][:, j:j + 1],
                                sig[:], op0=ALU.add, op1=ALU.mult)
                            nc.vector.scalar_tensor_tensor(
                                x2[:, j, :], tmp[:], INV, xt[:, j, :],
                                op0=ALU.mult, op1=ALU.add)
                            if j % 2 == 1:
                                ln_stats_pair(S1_2, S2_2, x2[:, j - 1, :],
                                              x2[:, j, :], wpool2, j // 2,
                                              NT // 2, "ln2")

                        proj_gated(NT, NT, ("Wo", y, True, f8),
                                   ("Wgo", y, True, f8), "bgo",
                                   wpool2, pj2ps, o_epi)
                    wpool2_cm.__exit__(None, None, None)
                    vresp_cm.__exit__(None, None, None)

              # ---- phase D: LN2 + MLP ----
              with tc.tile_pool(name="midp", bufs=1) as midp:
                  mid_bf = midp.tile([P, NF, S], bf16)
                  mid_f8 = midp.tile([P, NF, S], f8)
                  with tc.tile_pool(name="h2p", bufs=1) as h2p:
                      with tc.tile_pool(name="ln2tmp", bufs=1) as ln2tmp:
                          (h2_f8,) = ln_T(x2, "g2", "bt2", [(h2p, f8)],
                                          ln2tmp, ln2ps, "ln2",
                                          stats=(S1_2, S2_2))
                      ln2ps_cm.__exit__(None, None, None)

                      with (
                          tc.tile_pool(name="wmlp1", bufs=1) as wm1,
                          tc.tile_pool(name="m1ps", bufs=1,
                                       space="PSUM") as m1ps,
                      ):
                          def mid_epi(j, main, sig):
                              tmp = wm1.tile([P, S], fp32, tag="mid_tmp",
                                             name=f"mid_tmp_{j}", bufs=3)
                              nc.vector.scalar_tensor_tensor(
                                  tmp[:], main[:],
                                  bias["bin"][:, j:j + 1], sig[:],
                                  op0=ALU.add, op1=ALU.mult)
                              nc.scalar.activation(mid_bf[:, j, :], tmp[:],
                                                   AF.Gelu, scale=INV)
                              nc.scalar.activation(mid_f8[:, j, :], tmp[:],
                                                   AF.Gelu, scale=INV)

                          proj_gated(NT, NF, ("Win", h2_f8, True, f8),
                                     ("Wgin", h2_f8, True, f8), "bgin",
                                     wm1, m1ps, mid_epi)

                  with (
                      tc.tile_pool(name="wmlp2", bufs=1) as wm2,
                      tc.tile_pool(name="m2ps", bufs=1,
                                   space="PSUM") as m2ps,
                  ):
                      def out_epi(j, main, sig):
                          tmp = wm2.tile([P, S], fp32, tag="out_tmp",
                                         name=f"out_tmp_{j}", bufs=3)
                          nc.vector.scalar_tensor_tensor(
                              tmp[:], main[:], bias["bout"][:, j:j + 1],
                              sig[:], op0=ALU.add, op1=ALU.mult)
                          outf = wm2.tile([P, S], fp32, tag="out_f",
                                          name=f"out_f_{j}", bufs=3)
                          nc.vector.tensor_tensor(outf[:], tmp[:],
                                                  x2[:, j, :], op=ALU.add)
                          nc.sync.dma_start(
                              out_d.ap()[j * P:(j + 1) * P, :], outf[:])

                      proj_gated(NF, NT, ("Wout", mid_bf, False, bf16),
                                 ("Wgout", mid_f8, True, f8), "bgout",
                                 wm2, m2ps, out_epi, tchunk=32, wbufs=2)

    nc.compile()
    return nc


def _prep_shared_inputs(inputs):
    m = {}
    # fp8 weights stored at 64x (T-layout tiles)
    for nm, w in (("Wq", "W_q"), ("Wgq", "Wg_q"), ("Wk", "W_k"),
                  ("Wgk", "Wg_k"), ("Wo", "W_o"), ("Wgo", "Wg_o"),
                  ("Win", "W_in"), ("Wgin", "Wg_in"), ("Wgout", "Wg_out")):
        m[nm] = _w_tiled(np.asarray(inputs[w]), WS, _F8)
    # bf16 main-path mlp-out weights at 1x
    m["Wout"] = _w_tiled(np.asarray(inputs["W_out"]), 1.0, _BF)
    m["WvT"] = np.ascontiguousarray(
        (np.asarray(inputs["W_v"]).T * WS).astype(_F8))
    m["WgvT"] = np.ascontiguousarray(
        (np.asarray(inputs["Wg_v"]).T * WS).astype(_F8))
    m["bvrow"] = (np.asarray(inputs["b_v"]) * WS).astype(_BF).reshape(1, D)
    m["bgvrow"] = (np.asarray(inputs["bg_v"]) * WS).astype(_BF).reshape(1, D)
    for nm, bn, sc in (("bq", "b_q", WS), ("bgq", "bg_q", 1.0),
                       ("bk", "b_k", WS), ("bgk", "bg_k", 1.0),
                       ("bo", "b_o", WS), ("bgo", "bg_o", 1.0),
                       ("bin", "b_in", WS), ("bgin", "bg_in", 1.0),
                       ("bout", "b_out", 1.0), ("bgout", "bg_out", 1.0),
                       ("g1", "ln1_g", 1.0), ("bt1", "ln1_b", 1.0),
                       ("g2", "ln2_g", 1.0), ("bt2", "ln2_b", 1.0)):
        m[nm] = _b_cols(np.asarray(inputs[bn]), sc)
    return m


def _install_trace_shim():
    """Provide antenv.axon_hooks (NTFF profiling) if the image lacks it."""
    import contextlib
    import ctypes
    import types

    try:
        import antenv.axon_hooks  # noqa: F401
        return
    except ImportError:
        pass
    try:
        import antenv
    except ImportError:
        return
    so_path = "/opt/axon/libaxon_pjrt.so"
    try:
        lib = ctypes.CDLL(so_path)
    except OSError:
        return
    if not hasattr(lib, "axon_start_nrt_profile"):
        return
    lib.axon_start_nrt_profile.argtypes = [ctypes.POINTER(ctypes.c_int64),
                                           ctypes.c_size_t]
    lib.axon_start_nrt_profile.restype = ctypes.c_int64
    lib.axon_stop_nrt_profile.argtypes = [ctypes.c_char_p]
    lib.axon_stop_nrt_profile.restype = ctypes.c_int64

    @contextlib.contextmanager
    def hook(output_dir, device_ids):
        import jax

        jax.devices()
        if device_ids:
            ids = (ctypes.c_int64 * len(device_ids))(*device_ids)
            rc = lib.axon_start_nrt_profile(ids, len(device_ids))
        else:
            rc = lib.axon_start_nrt_profile(None, 0)
        if rc != 0:
            raise RuntimeError(f"axon_start_nrt_profile rc={rc}")
        try:
            yield
        finally:
            n = lib.axon_stop_nrt_profile(str(output_dir).encode())
            print(f"profile: {n} ntff file(s) in {output_dir}",
                  file=sys.stderr)

    mod = types.ModuleType("antenv.axon_hooks")
    mod.get_axon_ntff_profile_hook = lambda: hook
    mod.set_axon_ntff_profile_hook = lambda h: None
    sys.modules["antenv.axon_hooks"] = mod
    antenv.axon_hooks = mod


LAST_RESULTS = None


def kernel(_trace=False, **inputs):
    global _COMPILED, LAST_RESULTS
    from concourse import bass_utils

    if _trace:
        _install_trace_shim()

    if _COMPILED is None:
        _COMPILED = _build()
    nc = _COMPILED

    shared = _prep_shared_inputs(inputs)
    x = np.asarray(inputs["x"], dtype=np.float32)  # [B, T, D]
    in_maps = []
    for c in range(N_CORES):
        g, s = divmod(c, GROUP)
        xT_c = np.ascontiguousarray(x[g, s * S:(s + 1) * S, :].T)
        m = dict(shared)
        m["xT"] = xT_c
        in_maps.append(m)

    LAST_RESULTS = bass_utils.run_bass_kernel_spmd(
        nc, in_maps, core_ids=list(range(N_CORES)), trace=_trace)

    out = np.empty((B, T, D), dtype=np.float32)
    for c in range(N_CORES):
        g, s = divmod(c, GROUP)
        out[g, s * S:(s + 1) * S, :] = LAST_RESULTS.results[c]["outT"].T
    return out


# revision 36
# speedup vs baseline: 1.6091x; 1.0149x over previous
"""Trainium2 Bass kernel for nn_DGEBlock (dense transformer block with
MoE-gated linears), distributed over 8 NeuronCores.

Sharding: data-parallel over batch (2 groups of 4 cores) x sequence-parallel
over tokens within each batch (512 tokens per core). Weights are replicated.
Activations live feature-major ("T-layout": [d, tok]) in SBUF so projections
are lhsT=W^T-tile @ rhs=activation with no activation transposes. V is
projected in token-major (N-)layout directly so attention's PV matmuls need
no transposes either.

Precision scheme (fp8 DoubleRow halves the matmul count where used):
  - q/k/v/o projections: main+gate both fp8e4 DoubleRow (weights stored
    64x in e4m3; epilogues fold the 1/64 into activation scales).
  - MLP in/out: MAIN path stays bf16 (accuracy: errors there land on the
    residual stream through the widest matrices); GATE path fp8 DoubleRow
    (sigmoid compresses quantization noise).
  - Attention: At (exp logits) and V stored fp8; PV and the softmax
    denominator (Z) matmuls run DoubleRow over key-block pairs; QK stays
    bf16 (contraction is only 128 so DoubleRow can't help).
  - LayerNorm stats summed from fp8 copies via DoubleRow ones-matmuls
    (2048-way averaging makes this noise negligible).
The only collectives are two 4-rank AllGathers (V in fp8, K in bf16).
Output is returned token-sharded and reassembled on host.
"""

import sys

for _p in ("/opt/trn_rl_repo",):
    if _p not in sys.path:
        sys.path.append(_p)

import numpy as np
import ml_dtypes

# ---------------------------------------------------------------- constants
B = 2
T = 2048
D = 2048
H = 16
HD = 128
FF = 4 * D  # 8192
EPS = 1e-5

N_CORES = 8
GROUP = 4  # cores per batch group (sequence-parallel degree)
S = T // GROUP  # tokens per core = 512
P = 128
NT = D // P  # 16 feature tiles
NF = FF // P  # 64 hidden tiles
NKB = T // P  # 16 key blocks per batch
NPAIR = NKB // 2
ISCALE = 1.0 / float(np.sqrt(HD))

WS = 64.0  # fp8 weight pre-scale (keeps 0.02-std weights out of subnormals)
INV = 1.0 / WS
EXPSCALE = ISCALE / 16.0  # q and k are both stored fp8 at 4x

RG = [[0, 1, 2, 3], [4, 5, 6, 7]]

_BF = ml_dtypes.bfloat16
_F8 = ml_dtypes.float8_e4m3

_COMPILED = None


# ------------------------------------------------------------- host prep
def _w_tiled(W, scale, dt):
    """W [dout, din] -> [nj, 128, nt, 128] such that
    out[j, p, t, jc] == scale*W[j*128+jc, t*128+p]  (= W^T tile (t, j))."""
    dout, din = W.shape
    nj, nt = dout // P, din // P
    return np.ascontiguousarray(
        (W.reshape(nj, P, nt, P) * scale).transpose(0, 3, 2, 1).astype(dt)
    )


def _b_cols(b, scale=1.0):
    """b [dout] -> [128, nj] fp32: column j holds scale*b[j*128:(j+1)*128]."""
    nj = b.shape[0] // P
    return np.ascontiguousarray((b * scale).reshape(nj, P).T.astype(np.float32))


# ------------------------------------------------------------- device build
def _build():
    from concourse import bacc, tile, mybir

    fp32 = mybir.dt.float32
    bf16 = mybir.dt.bfloat16
    f8 = mybir.dt.float8e4
    AF = mybir.ActivationFunctionType
    ALU = mybir.AluOpType
    DR = mybir.MatmulPerfMode.DoubleRow

    nc = bacc.Bacc("TRN2", target_bir_lowering=False, debug=False,
                   num_devices=N_CORES)

    # ---- I/O tensors
    xT_d = nc.dram_tensor("xT", [D, S], fp32, kind="ExternalInput")
    wd = {}
    for nm in ("Wq", "Wgq", "Wk", "Wgk", "Wo", "Wgo"):
        wd[nm] = nc.dram_tensor(nm, [NT, P, NT, P], f8, kind="ExternalInput")
    wd["Win"] = nc.dram_tensor("Win", [NF, P, NT, P], f8,
                               kind="ExternalInput")
    wd["Wgin"] = nc.dram_tensor("Wgin", [NF, P, NT, P], f8,
                                kind="ExternalInput")
    wd["Wout"] = nc.dram_tensor("Wout", [NT, P, NF, P], bf16,
                                kind="ExternalInput")
    wd["Wgout"] = nc.dram_tensor("Wgout", [NT, P, NF, P], f8,
                                 kind="ExternalInput")
    # V projection runs in N-layout: plain W^T [din, dout] + bias rows
    wd["WvT"] = nc.dram_tensor("WvT", [D, D], f8, kind="ExternalInput")
    wd["WgvT"] = nc.dram_tensor("WgvT", [D, D], f8, kind="ExternalInput")
    bvrow_d = nc.dram_tensor("bvrow", [1, D], bf16, kind="ExternalInput")
    bgvrow_d = nc.dram_tensor("bgvrow", [1, D], bf16, kind="ExternalInput")
    bd = {}
    for nm in ("bq", "bgq", "bk", "bgk", "bo", "bgo",
               "bout", "bgout", "g1", "bt1", "g2", "bt2"):
        bd[nm] = nc.dram_tensor(nm, [P, NT], fp32, kind="ExternalInput")
    for nm in ("bin", "bgin"):
        bd[nm] = nc.dram_tensor(nm, [P, NF], fp32, kind="ExternalInput")
    out_d = nc.dram_tensor("outT", [D, S], fp32, kind="ExternalOutput")

    with tile.TileContext(nc) as tc:
        with (
            tc.tile_pool(name="const", bufs=1) as constp,
            tc.tile_pool(name="bias", bufs=1) as biasp,
            tc.tile_pool(name="rows", bufs=1) as rows,
            tc.tile_pool(name="dram", bufs=1, space="DRAM") as dramp,
        ):
            ones_col = constp.tile([P, 1], bf16)
            nc.vector.memset(ones_col[:], 1.0)
            ones_row = constp.tile([1, P], bf16)
            nc.vector.memset(ones_row[:], 1.0)
            # fp8 "ones" pair for DoubleRow contractions with unit weights;
            # [P, 2, 16] so the pair-step is 16B (DoubleRow AP constraint)
            ones2 = constp.tile([P, 2, 16], f8)
            nc.vector.memset(ones2[:], 1.0)
            eps_t = constp.tile([1, 1], fp32)
            nc.vector.memset(eps_t[:], EPS)

            bias = {}
            for nm in bd:
                ncols = NF if nm in ("bin", "bgin") else NT
                btile = biasp.tile([P, ncols], fp32, name=f"bias_{nm}")
                nc.sync.dma_start(btile[:], bd[nm].ap())
                bias[nm] = btile

            # ---------- helpers ----------
            def ln_stats_pair(S1, S2, src0, src1, tmpool, pi, npair, name):
                """Accumulate sum/sum-sq of one feature-tile pair into the
                S1/S2 psums via fp8 DoubleRow ones-matmuls."""
                xp = tmpool.tile([P, 2, S], f8, tag="ln_x8", bufs=2,
                                 name=f"{name}_x8_{pi}")
                sp = tmpool.tile([P, 2, S], f8, tag="ln_sq", bufs=2,
                                 name=f"{name}_sq_{pi}")
                for i, srct in enumerate((src0, src1)):
                    nc.vector.tensor_copy(xp[:, i, :], srct)
                    nc.scalar.activation(sp[:, i, :], srct, AF.Square)
                nc.tensor.matmul(S1[:], ones2[:, :, 0:1], xp[:],
                                 start=(pi == 0), stop=(pi == npair - 1),
                                 perf_mode=DR)
                nc.tensor.matmul(S2[:], ones2[:, :, 0:1], sp[:],
                                 start=(pi == 0), stop=(pi == npair - 1),
                                 perf_mode=DR)

            def ln_T(src, gname, bname, outs, tmpool, psln, name,
                     stats=None):
                """LayerNorm over the feature dim of a T-layout activation.

                src: SBUF tile [128, NT, S] fp32.  outs: list of
                (pool, dtype) -> returns one [128, NT, S] tile per entry.
                Stats via fp8 DoubleRow ones-matmuls (contract over
                partitions) unless passed precomputed; per-token scale/shift
                rows are broadcast to [128, S] via rank-1 matmuls.  The
                apply chain alternates Vector/GpSimd per tile to halve the
                serial latency.
                """
                if stats is None:
                    S1 = psln.tile([1, S], fp32, name=f"{name}_S1",
                                   tag="ln_S1")
                    S2 = psln.tile([1, S], fp32, name=f"{name}_S2",
                                   tag="ln_S2")
                    for pi in range(NT // 2):
                        t = 2 * pi
                        ln_stats_pair(S1, S2, src[:, t, :], src[:, t + 1, :],
                                      tmpool, pi, NT // 2, name)
                else:
                    S1, S2 = stats

                def row(nm, dt=fp32):
                    return rows.tile([1, S], dt, name=f"{name}_{nm}",
                                     tag=f"ln_{nm}")

                mean = row("mean")
                nc.vector.tensor_scalar_mul(mean[:], S1[:], 1.0 / D)
                m2 = row("m2")
                nc.vector.tensor_scalar_mul(m2[:], S2[:], 1.0 / D)
                msq = row("msq")
                nc.vector.tensor_tensor(msq[:], mean[:], mean[:],
                                        op=ALU.mult)
                var = row("var")
                nc.vector.tensor_tensor(var[:], m2[:], msq[:],
                                        op=ALU.subtract)
                # rstd = sqrt(1/(var+eps)): keeps the whole prefix on DVE,
                # single hop to ACT which emits bf16 directly
                veps = row("veps")
                nc.vector.tensor_scalar_add(veps[:], var[:], EPS)
                iv = row("iv")
                nc.vector.reciprocal(iv[:], veps[:])
                rstd_bf = row("rstdbf", bf16)
                nc.scalar.activation(rstd_bf[:], iv[:], AF.Sqrt)
                mr_bf = row("mrbf", bf16)
                nc.vector.tensor_tensor(mr_bf[:], mean[:], rstd_bf[:],
                                        op=ALU.mult)
                Ab_p = psln.tile([P, S], fp32, name=f"{name}_Abp",
                                 tag="ln_Abp")
                nc.tensor.matmul(Ab_p[:], ones_row[:], rstd_bf[:])
                Bb_p = psln.tile([P, S], fp32, name=f"{name}_Bbp",
                                 tag="ln_Bbp")
                nc.tensor.matmul(Bb_p[:], ones_row[:], mr_bf[:])
                # per-engine operand copies so the Vector/GpSimd apply
                # streams don't contend on the same SBUF tiles
                Ab_v = tmpool.tile([P, S], fp32, name=f"{name}_Abv")
                nc.vector.tensor_copy(Ab_v[:], Ab_p[:])
                Bb_v = tmpool.tile([P, S], fp32, name=f"{name}_Bbv")
                nc.vector.tensor_copy(Bb_v[:], Bb_p[:])
                Ab_g = tmpool.tile([P, S], fp32, name=f"{name}_Abg")
                nc.scalar.copy(Ab_g[:], Ab_p[:])
                Bb_g = tmpool.tile([P, S], fp32, name=f"{name}_Bbg")
                nc.scalar.copy(Bb_g[:], Bb_p[:])
                hs = [pool.tile([P, NT, S], dt, name=f"{name}_h{i}")
                      for i, (pool, dt) in enumerate(outs)]
                for t in range(NT):
                    if t % 2 == 0:
                        eng, Ab, Bb, tg = nc.vector, Ab_v, Bb_v, "v"
                    else:
                        eng, Ab, Bb, tg = nc.gpsimd, Ab_g, Bb_g, "g"
                    tmp = tmpool.tile([P, S], fp32, name=f"{name}_t0_{t}",
                                      tag=f"ln_t0{tg}", bufs=2)
                    eng.tensor_tensor(tmp[:], src[:, t, :], Ab[:],
                                      op=ALU.mult)
                    tmp2 = tmpool.tile([P, S], fp32, name=f"{name}_t1_{t}",
                                       tag=f"ln_t1{tg}", bufs=2)
                    eng.tensor_tensor(tmp2[:], tmp[:], Bb[:],
                                      op=ALU.subtract)
                    for h in hs:
                        nc.scalar.activation(h[:, t, :], tmp2[:], AF.Identity,
                                             bias=bias[bname][:, t:t + 1],
                                             scale=bias[gname][:, t:t + 1])
                return hs

            def prefetch_w(wname, j, tchunk, wpool, tag, wdt, nchunk=1,
                           wbufs=3):
                """Early-load weight chunks for (wname, j) on the Vector
                DMA queue so a phase's first matmuls don't wait behind the
                previous phase's self-paced sync-queue weight stream."""
                out = {}
                for ci in range(nchunk):
                    wt = wpool.tile([P, tchunk, P], wdt, tag=tag,
                                    name=f"w_{wname}_{j}_{ci}", bufs=wbufs)
                    nc.scalar.dma_start(
                        wt[:],
                        wd[wname].ap()[j, :,
                                       ci * tchunk:(ci + 1) * tchunk, :])
                    out[(j, ci)] = wt
                return out

            def proj_gated(nt, nj, main_spec, gate_spec, bgname, wpool,
                           pspool, epilogue, tchunk=None, wbufs=3,
                           pf_main=None, pf_gate=None):
                """Gated projection in T-layout.  spec = (wname, src, dr,
                wdtype).  dr=True runs fp8 DoubleRow over k-tile pairs."""
                if tchunk is None:
                    tchunk = nt
                nchunk = nt // tchunk
                wname, src_m, dr_m, dt_m = main_spec
                wgname, src_g, dr_g, dt_g = gate_spec
                for j in range(nj):
                    main = pspool.tile([P, S], fp32, name=f"{wname}_m{j}",
                                       tag="pj_main", bufs=2)
                    gate = pspool.tile([P, S], fp32, name=f"{wname}_g{j}",
                                       tag="pj_gate", bufs=2)

                    def path(acc, wnm, src, dr, wdt, tag, pf):
                        for ci in range(nchunk):
                            if pf and (j, ci) in pf:
                                wt = pf[(j, ci)]
                            else:
                                wt = wpool.tile([P, tchunk, P], wdt,
                                                tag=tag,
                                                name=f"w_{wnm}_{j}_{ci}",
                                                bufs=wbufs)
                                nc.sync.dma_start(
                                    wt[:],
                                    wd[wnm].ap()[
                                        j, :,
                                        ci * tchunk:(ci + 1) * tchunk, :])
                            if dr:
                                for pi in range(tchunk // 2):
                                    t = ci * tchunk + 2 * pi
                                    nc.tensor.matmul(
                                        acc[:], wt[:, 2 * pi:2 * pi + 2, :],
                                        src[:, t:t + 2, :],
                                        start=(t == 0), stop=(t == nt - 2),
                                        perf_mode=DR)
                            else:
                                for ti in range(tchunk):
                                    t = ci * tchunk + ti
                                    nc.tensor.matmul(
                                        acc[:], wt[:, ti, :], src[:, t, :],
                                        start=(t == 0), stop=(t == nt - 1))

                    path(main, wname, src_m, dr_m, dt_m, "wmain", pf_main)
                    path(gate, wgname, src_g, dr_g, dt_g, "wgate", pf_gate)
                    sig = wpool.tile([P, S], bf16, tag="sig",
                                     name=f"sig_{wname}_{j}", bufs=3)
                    nc.scalar.activation(sig[:], gate[:], AF.Sigmoid,
                                         bias=bias[bgname][:, j:j + 1],
                                         scale=(INV if dr_g else 1.0))
                    epilogue(j, main, sig)

            # x2 outlives phases A-C (used by LN2 + MLP residual)
            with tc.tile_pool(name="x2p", bufs=1) as x2p:
              with tc.tile_pool(name="xt", bufs=1) as xtp:
                xt = xtp.tile([P, NT, S], fp32)
                xT_v = xT_d.ap().rearrange("(t p) s -> t p s", p=P)
                for t in range(NT):
                    nc.sync.dma_start(xt[:, t, :], xT_v[t])

                vN_bounce = dramp.tile([S, D], f8)
                k_bounce = dramp.tile([D, S], f8)
                vgN = dramp.tile([GROUP * S, D], f8)
                kg = dramp.tile([GROUP * D, S], f8)

                with tc.tile_pool(name="yp", bufs=1) as ypool:
                  with tc.tile_pool(name="qp", bufs=1) as qpool:
                    q = qpool.tile([P, NT, S], f8)
                    vresp_cm = tc.tile_pool(name="vres", bufs=1)
                    vresp = vresp_cm.__enter__()
                    # V resident [k-part, kb, d] fp8; loads issued right
                    # after the V AllGather (ahead of the K collective on
                    # the GpSimd queue) so they overlap the Q projection.
                    Vt = vresp.tile([P, NKB, D], f8)

                    with tc.tile_pool(name="hq", bufs=1) as hqp:
                      # Phase order: K-proj -> CC(K) -> V-proj -> CC(V)
                      # -> Q-proj, so each AllGather overlaps the next
                      # projection's matmul stream.  The weight pool opens
                      # before LN1 so the first K chunk prefetches during
                      # the x load.
                      with tc.tile_pool(name="wproj", bufs=1) as wpool:
                        pf_k = prefetch_w("Wk", 0, NT, wpool, "wmain", f8,
                                          wbufs=4)
                        pf_gk = prefetch_w("Wgk", 0, NT, wpool, "wgate", f8,
                                           wbufs=4)
                        with (
                            tc.tile_pool(name="ln1tmp", bufs=1) as ln1tmp,
                            tc.tile_pool(name="ln1ps", bufs=1,
                                         space="PSUM") as ln1ps,
                        ):
                            (h1,) = ln_T(xt, "g1", "bt1", [(hqp, f8)],
                                         ln1tmp, ln1ps, "ln1")

                        if True:
                          with tc.tile_pool(name="pjps", bufs=1,
                                            space="PSUM") as pjps:
                            def k_epi(j, main, sig):
                                kv64 = wpool.tile([P, S], fp32, tag="kv64",
                                                  name=f"kv64_{j}", bufs=3)
                                nc.vector.scalar_tensor_tensor(
                                    kv64[:], main[:],
                                    bias["bk"][:, j:j + 1],
                                    sig[:], op0=ALU.add, op1=ALU.mult)
                                kv = wpool.tile([P, S], f8, tag="kv_out",
                                                name=f"kv_k_{j}", bufs=3)
                                nc.vector.tensor_scalar_mul(kv[:], kv64[:],
                                                            1.0 / 16.0)
                                nc.scalar.dma_start(
                                    k_bounce[j * P:(j + 1) * P, :], kv[:])

                            proj_gated(NT, NT, ("Wk", h1, True, f8),
                                       ("Wgk", h1, True, f8), "bgk",
                                       wpool, pjps, k_epi, wbufs=4,
                                       pf_main=pf_k, pf_gate=pf_gk)

                            nc.gpsimd.collective_compute(
                                "AllGather", ALU.bypass, ins=[k_bounce[:]],
                                outs=[kg[:]], replica_groups=RG)

                          # ---- V projection, N-layout, fp8 DoubleRow ----
                          with (
                            tc.tile_pool(name="wv", bufs=1) as wvp,
                            tc.tile_pool(name="vps", bufs=1,
                                         space="PSUM") as vps,
                          ):
                            TC = NT // 2
                            bvrow = wvp.tile([1, D], bf16, name="bvrow")
                            nc.sync.dma_start(bvrow[:], bvrow_d.ap())
                            bgvrow = wvp.tile([1, D], bf16, name="bgvrow")
                            nc.sync.dma_start(bgvrow[:], bgvrow_d.ap())

                            def v_wtiles(n, ci, queue):
                                wvt = wvp.tile([P, TC, 4 * P], f8,
                                               tag="wv", bufs=2,
                                               name=f"wv_{n}_{ci}")
                                wgvt = wvp.tile([P, TC, 4 * P], f8,
                                                tag="wgv", bufs=2,
                                                name=f"wgv_{n}_{ci}")
                                for ti in range(TC):
                                    t = ci * TC + ti
                                    queue.dma_start(
                                        wvt[:, ti, :],
                                        wd["WvT"].ap()[t * P:(t + 1) * P,
                                                       n * S:(n + 1) * S])
                                    queue.dma_start(
                                        wgvt[:, ti, :],
                                        wd["WgvT"].ap()[
                                            t * P:(t + 1) * P,
                                            n * S:(n + 1) * S])
                                return wvt, wgvt

                            # first V chunk prefetched on the ACT queue
                            # (runs during K-proj, ahead of CC(K) traffic)
                            pf_v = v_wtiles(0, 0, nc.scalar)
                            # first Q chunk likewise
                            pf_q = prefetch_w("Wq", 0, NT, wpool, "wmain",
                                              f8, wbufs=4)
                            pf_gq = prefetch_w("Wgq", 0, NT, wpool,
                                               "wgate", f8, wbufs=4)
                            for n in range(4):
                                vmain = [vps.tile([P, S], fp32,
                                                  tag="v_main", bufs=4,
                                                  name=f"vm_{n}_{m}")
                                         for m in range(4)]
                                vgate = [vps.tile([P, S], fp32,
                                                  tag="v_gate", bufs=4,
                                                  name=f"vg_{n}_{m}")
                                         for m in range(4)]
                                for ci in range(2):
                                    if n == 0 and ci == 0:
                                        wvt, wgvt = pf_v
                                    else:
                                        wvt, wgvt = v_wtiles(n, ci, nc.sync)
                                    for m in range(4):
                                        for pi in range(TC // 2):
                                            t = ci * TC + 2 * pi
                                            nc.tensor.matmul(
                                                vmain[m][:],
                                                h1[:, t:t + 2,
                                                   m * P:(m + 1) * P],
                                                wvt[:, 2 * pi:2 * pi + 2, :],
                                                start=(t == 0), stop=False,
                                                perf_mode=DR)
                                        for pi in range(TC // 2):
                                            t = ci * TC + 2 * pi
                                            nc.tensor.matmul(
                                                vgate[m][:],
                                                h1[:, t:t + 2,
                                                   m * P:(m + 1) * P],
                                                wgvt[:, 2 * pi:2 * pi + 2, :],
                                                start=(t == 0), stop=False,
                                                perf_mode=DR)
                                for m in range(4):
                                    nc.tensor.matmul(
                                        vmain[m][:], ones_row[:],
                                        bvrow[:, n * S:(n + 1) * S],
                                        start=False, stop=True)
                                    nc.tensor.matmul(
                                        vgate[m][:], ones_row[:],
                                        bgvrow[:, n * S:(n + 1) * S],
                                        start=False, stop=True)
                                    vsig = wvp.tile([P, S], bf16,
                                                    tag="vsig", bufs=3,
                                                    name=f"vsig_{n}_{m}")
                                    nc.scalar.activation(vsig[:],
                                                         vgate[m][:],
                                                         AF.Sigmoid,
                                                         scale=INV)
                                    vout = wvp.tile([P, S], f8,
                                                    tag="vout", bufs=3,
                                                    name=f"vout_{n}_{m}")
                                    nc.vector.scalar_tensor_tensor(
                                        vout[:], vmain[m][:], INV, vsig[:],
                                        op0=ALU.mult, op1=ALU.mult)
                                    nc.scalar.dma_start(
                                        vN_bounce[m * P:(m + 1) * P,
                                                  n * S:(n + 1) * S],
                                        vout[:])

                            nc.gpsimd.collective_compute(
                                "AllGather", ALU.bypass, ins=[vN_bounce[:]],
                                outs=[vgN[:]], replica_groups=RG)
                            for kb in range(NKB):
                                nc.gpsimd.dma_start(
                                    Vt[:, kb, :],
                                    vgN[kb * P:(kb + 1) * P, :])

                          # ---- Q projection ----
                          with tc.tile_pool(name="pjps2", bufs=1,
                                            space="PSUM") as pjps2:
                            def q_epi(j, main, sig):
                                q64 = wpool.tile([P, S], fp32, tag="kv64",
                                                 name=f"q64_{j}", bufs=3)
                                nc.vector.scalar_tensor_tensor(
                                    q64[:], main[:],
                                    bias["bq"][:, j:j + 1],
                                    sig[:], op0=ALU.add, op1=ALU.mult)
                                nc.vector.tensor_scalar_mul(
                                    q[:, j, :], q64[:], 1.0 / 16.0)

                            proj_gated(NT, NT, ("Wq", h1, True, f8),
                                       ("Wgq", h1, True, f8), "bgq",
                                       wpool, pjps2, q_epi, wbufs=4,
                                       pf_main=pf_q, pf_gate=pf_gq)

                    # o-proj weight pool opened BEFORE the attention pools
                    # so its SBUF range is disjoint from them -- its first
                    # weight DMAs then prefetch during attention instead of
                    # waiting for attention buffers to free.
                    wpool2_cm = tc.tile_pool(name="wproj2", bufs=1)
                    wpool2 = wpool2_cm.__enter__()

                    # ---- phase B: attention ----
                    with (
                        tc.tile_pool(name="kstream", bufs=2) as kpool,
                        tc.tile_pool(name="apool", bufs=4) as apool,
                        tc.tile_pool(name="atps", bufs=1,
                                     space="PSUM") as atps,
                    ):
                        y = ypool.tile([P, NT, S], f8)

                        head_state = {}

                        def finalize_head(h, Zp_h, Yp_h):
                            urow = apool.tile([1, S], fp32, name=f"u_{h}",
                                              tag="urow", bufs=2)
                            nc.vector.reciprocal(urow[:], Zp_h[:])
                            ubf = apool.tile([1, S], bf16, name=f"ubf_{h}",
                                             tag="ubf", bufs=2)
                            nc.vector.tensor_copy(ubf[:], urow[:])
                            Up = atps.tile([P, S], fp32, name=f"Up_{h}",
                                           tag="logits", bufs=4)
                            nc.tensor.matmul(Up[:], ones_row[:], ubf[:])
                            Us = apool.tile([P, S], bf16, tag="Us",
                                            name=f"Us_{h}")
                            nc.vector.tensor_copy(Us[:], Up[:])
                            nc.vector.tensor_tensor(y[:, h, :], Yp_h[:],
                                                    Us[:], op=ALU.mult)

                        for hh in range(H):
                            Kh = kpool.tile([P, NKB * P], f8, tag="Kh",
                                            name=f"Kh_{hh}")
                            for s_ in range(GROUP):
                                nc.gpsimd.dma_start(
                                    Kh[:, s_ * S:(s_ + 1) * S],
                                    kg[s_ * D + hh * P:
                                       s_ * D + (hh + 1) * P, :])
                            Zp = atps.tile([1, S], fp32, name=f"Z_{hh}",
                                           tag="Zp", bufs=2)
                            Yp = atps.tile([P, S], fp32, name=f"Y_{hh}",
                                           tag="Yp", bufs=2)
                            ats = {}

                            def do_pair(pi, hh=hh, Kh=Kh, ats=ats):
                                At2 = apool.tile([P, 2, S], f8, tag="At2",
                                                 name=f"At2_{hh}_{pi}",
                                                 bufs=4)
                                for i in range(2):
                                    kb = 2 * pi + i
                                    Lp = atps.tile([P, S], fp32,
                                                   name=f"L_{hh}_{kb}",
                                                   tag="logits", bufs=4)
                                    nc.tensor.matmul(
                                        Lp[:], Kh[:, kb * P:(kb + 1) * P],
                                        q[:, hh, :])
                                    nc.scalar.activation(At2[:, i, :], Lp[:],
                                                         AF.Exp,
                                                         scale=EXPSCALE)
                                ats[pi] = At2

                            do_pair(0)
                            do_pair(1)
                            for pi in range(NPAIR):
                                if pi + 2 < NPAIR:
                                    do_pair(pi + 2)
                                At2 = ats.pop(pi)
                                nc.tensor.matmul(Zp[:], ones2[:, :, 0:1],
                                                 At2[:],
                                                 start=(pi == 0),
                                                 stop=(pi == NPAIR - 1),
                                                 perf_mode=DR)
                                nc.tensor.matmul(
                                    Yp[:],
                                    Vt[:, 2 * pi:2 * pi + 2,
                                       hh * P:(hh + 1) * P],
                                    At2[:],
                                    start=(pi == 0),
                                    stop=(pi == NPAIR - 1),
                                    perf_mode=DR)
                                if pi == 2 and hh > 0:
                                    finalize_head(hh - 1,
                                                  *head_state[hh - 1])
                            head_state[hh] = (Zp, Yp)
                        finalize_head(H - 1, *head_state[H - 1])

                    # ---- phase C: o-proj + residual (LN2 stats inline) ---
                    x2 = x2p.tile([P, NT, S], fp32, name="x2")
                    ln2ps_cm = tc.tile_pool(name="ln2ps", bufs=1,
                                            space="PSUM")
                    ln2ps = ln2ps_cm.__enter__()
                    S1_2 = ln2ps.tile([1, S], fp32, name="ln2_S1",
                                      tag="ln_S1")
                    S2_2 = ln2ps.tile([1, S], fp32, name="ln2_S2",
                                      tag="ln_S2")
                    with tc.tile_pool(name="pj2ps", bufs=1,
                                      space="PSUM") as pj2ps:
                        def o_epi(j, main, sig):
                            tmp = wpool2.tile([P, S], fp32, tag="o_tmp",
                                              name=f"o_tmp_{j}", bufs=3)
                            nc.vector.scalar_tensor_tensor(
                                tmp[:], main[:], bias["bo"][:, j:j + 1],
                                sig[:], op0=ALU.add, op1=ALU.mult)
                            nc.vector.scalar_tensor_tensor(
                                x2[:, j, :], tmp[:], INV, xt[:, j, :],
                                op0=ALU.mult, op1=ALU.add)
                            if j % 2 == 1:
                                ln_stats_pair(S1_2, S2_2, x2[:, j - 1, :],
                                              x2[:, j, :], wpool2, j // 2,
                                              NT // 2, "ln2")

                        proj_gated(NT, NT, ("Wo", y, True, f8),
                                   ("Wgo", y, True, f8), "bgo",
                                   wpool2, pj2ps, o_epi)
                    wpool2_cm.__exit__(None, None, None)
                    vresp_cm.__exit__(None, None, None)

              # ---- phase D: LN2 + MLP ----
              with tc.tile_pool(name="midp", bufs=1) as midp:
                  mid_bf = midp.tile([P, NF, S], bf16)
                  mid_f8 = midp.tile([P, NF, S], f8)
                  with tc.tile_pool(name="h2p", bufs=1) as h2p:
                      with tc.tile_pool(name="ln2tmp", bufs=1) as ln2tmp:
                          (h2_f8,) = ln_T(x2, "g2", "bt2", [(h2p, f8)],
                                          ln2tmp, ln2ps, "ln2",
                                          stats=(S1_2, S2_2))
                      ln2ps_cm.__exit__(None, None, None)

                      # out-proj weight pool opened early: its first
                      # chunks prefetch on the Vector queue during LN2 /
                      # the in-projection.
                      wm2_cm = tc.tile_pool(name="wmlp2", bufs=1)
                      wm2 = wm2_cm.__enter__()
                      pf_om = prefetch_w("Wout", 0, 32, wm2, "wmain",
                                         bf16, nchunk=2, wbufs=2)
                      pf_og = prefetch_w("Wgout", 0, 32, wm2, "wgate",
                                         f8, nchunk=2, wbufs=2)

                      with (
                          tc.tile_pool(name="wmlp1", bufs=1) as wm1,
                          tc.tile_pool(name="m1ps", bufs=1,
                                       space="PSUM") as m1ps,
                      ):
                          def mid_epi(j, main, sig):
                              tmp = wm1.tile([P, S], fp32, tag="mid_tmp",
                                             name=f"mid_tmp_{j}", bufs=3)
                              nc.vector.scalar_tensor_tensor(
                                  tmp[:], main[:],
                                  bias["bin"][:, j:j + 1], sig[:],
                                  op0=ALU.add, op1=ALU.mult)
                              nc.scalar.activation(mid_bf[:, j, :], tmp[:],
                                                   AF.Gelu, scale=INV)
                              nc.scalar.activation(mid_f8[:, j, :], tmp[:],
                                                   AF.Gelu, scale=INV)

                          proj_gated(NT, NF, ("Win", h2_f8, True, f8),
                                     ("Wgin", h2_f8, True, f8), "bgin",
                                     wm1, m1ps, mid_epi, wbufs=2)

                      with tc.tile_pool(name="m2ps", bufs=1,
                                        space="PSUM") as m2ps:
                          def out_epi(j, main, sig):
                              tmp = wm2.tile([P, S], fp32, tag="out_tmp",
                                             name=f"out_tmp_{j}", bufs=3)
                              nc.vector.scalar_tensor_tensor(
                                  tmp[:], main[:], bias["bout"][:, j:j + 1],
                                  sig[:], op0=ALU.add, op1=ALU.mult)
                              outf = wm2.tile([P, S], fp32, tag="out_f",
                                              name=f"out_f_{j}", bufs=3)
                              nc.vector.tensor_tensor(outf[:], tmp[:],
                                                      x2[:, j, :],
                                                      op=ALU.add)
                              nc.sync.dma_start(
                                  out_d.ap()[j * P:(j + 1) * P, :],
                                  outf[:])

                          proj_gated(NF, NT,
                                     ("Wout", mid_bf, False, bf16),
                                     ("Wgout", mid_f8, True, f8), "bgout",
                                     wm2, m2ps, out_epi, tchunk=32,
                                     wbufs=2, pf_main=pf_om,
                                     pf_gate=pf_og)
                      wm2_cm.__exit__(None, None, None)

    nc.compile()
    return nc


def _prep_shared_inputs(inputs):
    m = {}
    # fp8 weights stored at 64x (T-layout tiles)
    for nm, w in (("Wq", "W_q"), ("Wgq", "Wg_q"), ("Wk", "W_k"),
                  ("Wgk", "Wg_k"), ("Wo", "W_o"), ("Wgo", "Wg_o"),
                  ("Win", "W_in"), ("Wgin", "Wg_in"), ("Wgout", "Wg_out")):
        m[nm] = _w_tiled(np.asarray(inputs[w]), WS, _F8)
    # bf16 main-path mlp-out weights at 1x
    m["Wout"] = _w_tiled(np.asarray(inputs["W_out"]), 1.0, _BF)
    m["WvT"] = np.ascontiguousarray(
        (np.asarray(inputs["W_v"]).T * WS).astype(_F8))
    m["WgvT"] = np.ascontiguousarray(
        (np.asarray(inputs["Wg_v"]).T * WS).astype(_F8))
    m["bvrow"] = (np.asarray(inputs["b_v"]) * WS).astype(_BF).reshape(1, D)
    m["bgvrow"] = (np.asarray(inputs["bg_v"]) * WS).astype(_BF).reshape(1, D)
    for nm, bn, sc in (("bq", "b_q", WS), ("bgq", "bg_q", 1.0),
                       ("bk", "b_k", WS), ("bgk", "bg_k", 1.0),
                       ("bo", "b_o", WS), ("bgo", "bg_o", 1.0),
                       ("bin", "b_in", WS), ("bgin", "bg_in", 1.0),
                       ("bout", "b_out", 1.0), ("bgout", "bg_out", 1.0),
                       ("g1", "ln1_g", 1.0), ("bt1", "ln1_b", 1.0),
                       ("g2", "ln2_g", 1.0), ("bt2", "ln2_b", 1.0)):
        m[nm] = _b_cols(np.asarray(inputs[bn]), sc)
    return m


def _install_trace_shim():
    """Provide antenv.axon_hooks (NTFF profiling) if the image lacks it."""
    import contextlib
    import ctypes
    import types

    try:
        import antenv.axon_hooks  # noqa: F401
        return
    except ImportError:
        pass
    try:
        import antenv
    except ImportError:
        return
    so_path = "/opt/axon/libaxon_pjrt.so"
    try:
        lib = ctypes.CDLL(so_path)
    except OSError:
        return
    if not hasattr(lib, "axon_start_nrt_profile"):
        return
    lib.axon_start_nrt_profile.argtypes = [ctypes.POINTER(ctypes.c_int64),
                                           ctypes.c_size_t]
    lib.axon_start_nrt_profile.restype = ctypes.c_int64
    lib.axon_stop_nrt_profile.argtypes = [ctypes.c_char_p]
    lib.axon_stop_nrt_profile.restype = ctypes.c_int64

    @contextlib.contextmanager
    def hook(output_dir, device_ids):
        import jax

        jax.devices()
        if device_ids:
            ids = (ctypes.c_int64 * len(device_ids))(*device_ids)
            rc = lib.axon_start_nrt_profile(ids, len(device_ids))
        else:
            rc = lib.axon_start_nrt_profile(None, 0)
        if rc != 0:
            raise RuntimeError(f"axon_start_nrt_profile rc={rc}")
        try:
            yield
        finally:
            n = lib.axon_stop_nrt_profile(str(output_dir).encode())
            print(f"profile: {n} ntff file(s) in {output_dir}",
                  file=sys.stderr)

    mod = types.ModuleType("antenv.axon_hooks")
    mod.get_axon_ntff_profile_hook = lambda: hook
    mod.set_axon_ntff_profile_hook = lambda h: None
    sys.modules["antenv.axon_hooks"] = mod
    antenv.axon_hooks = mod


LAST_RESULTS = None


def kernel(_trace=False, **inputs):
    global _COMPILED, LAST_RESULTS
    from concourse import bass_utils

    if _trace:
        _install_trace_shim()

    if _COMPILED is None:
        _COMPILED = _build()
    nc = _COMPILED

    shared = _prep_shared_inputs(inputs)
    x = np.asarray(inputs["x"], dtype=np.float32)  # [B, T, D]
    in_maps = []
    for c in range(N_CORES):
        g, s = divmod(c, GROUP)
        xT_c = np.ascontiguousarray(x[g, s * S:(s + 1) * S, :].T)
        m = dict(shared)
        m["xT"] = xT_c
        in_maps.append(m)

    LAST_RESULTS = bass_utils.run_bass_kernel_spmd(
        nc, in_maps, core_ids=list(range(N_CORES)), trace=_trace)

    out = np.empty((B, T, D), dtype=np.float32)
    for c in range(N_CORES):
        g, s = divmod(c, GROUP)
        out[g, s * S:(s + 1) * S, :] = LAST_RESULTS.results[c]["outT"].T
    return out
